# revision 1
# baseline (speedup 1.0000x reference)
"""Trainium2 Bass kernel for nn_CLNF_54769422959177.

Computes (dp, dw) where dp = vf(p) (4-layer VectorField MLP forward) and
dw = -vjp(vf, p)(w), data-parallel over 8 NeuronCores.

Layout: row-major tiles [128 rows, GBLK, 128 features]; fp16 matmuls with
stationary = PE-transposed activations and moving = weights; biases preloaded
into PSUM via K=1 ones-outer-product matmuls; LayerNorm stats via segmented
reduce + ACT Square; rstd = exp(-0.5*ln(var+eps)) (single natural_log_exp ACT
table set); backward LN/softplus chain fused into per-block
scalar_tensor_tensor/tensor_scalar ops; gamma/beta, the sin time-embedding,
the gdy mean-subtraction, and the h4->y / backward-entry matmul pairs are all
folded into host-precomputed weights. Emission is a rolling 2-wide lockstep
software pipeline (fwd of pair k overlaps bwd tails of earlier pairs).
"""

import numpy as np
from contextlib import ExitStack

import concourse.bass as bass
from concourse import bacc
import concourse.tile as tile
from concourse import mybir
from concourse.bass_utils import run_bass_kernel_spmd

B, D, H, L = 131072, 128, 128, 4
NCORES = 8
LN_EPS = 1e-5
FP16 = mybir.dt.float16
F32 = mybir.dt.float32
AF = mybir.ActivationFunctionType
OP = mybir.AluOpType


def _bview(ap, n):
    """Broadcast a [128, G, 1] stat AP along a new innermost dim of size n."""
    return bass.AP(
        tensor=ap.tensor,
        offset=ap.offset,
        ap=[list(ap.ap[0]), list(ap.ap[1]), [0, n]],
    )


def _copy(nc, out, in_):
    """Copy/cast via TENSOR_SCALAR encoding (TensorCopy's TR struct allows only
    one sync-wait slot in walrus codegen)."""
    nc.vector.tensor_scalar(
        out=out, in0=in_, scalar1=1.0, scalar2=None, op0=mybir.AluOpType.mult
    )


def _emit(nc, R, GBLK):
    """Emit the per-core program: R rows, blocks of 128 rows, GBLK blocks/group."""
    NG = R // (GBLK * 128)
    assert NG * GBLK * 128 == R

    p_in = nc.dram_tensor("p", [R, D], F32, kind="ExternalInput")
    w_in = nc.dram_tensor("w", [R, D], F32, kind="ExternalInput")
    # moving weights, fwd: [K, 6, N] = {W_in.T, Wg0.T, Wg1.T, Wg2.T, Wg3.T, W_out.T}
    wf_in = nc.dram_tensor("wf", [128, 6, 128], FP16, kind="ExternalInput")
    # moving weights, bwd: {-W_out, Wc3, Wc2, Wc1, Wc0, W_in}
    wb_in = nc.dram_tensor("wb", [128, 6, 128], FP16, kind="ExternalInput")
    cb_in = nc.dram_tensor("cb", [1, 6, 128], FP16, kind="ExternalInput")
    ones_in = nc.dram_tensor("ones1", [1, 128], FP16, kind="ExternalInput")
    id_in = nc.dram_tensor("ident", [128, 128], FP16, kind="ExternalInput")
    dp_out = nc.dram_tensor("dp", [R, D], F32, kind="ExternalOutput")
    dw_out = nc.dram_tensor("dw", [R, D], F32, kind="ExternalOutput")

    pv = p_in[:, :].rearrange("(g b p) d -> g p b d", p=128, b=GBLK)
    wv = w_in[:, :].rearrange("(g b p) d -> g p b d", p=128, b=GBLK)
    dpv = dp_out[:, :].rearrange("(g b p) d -> g p b d", p=128, b=GBLK)
    dwv = dw_out[:, :].rearrange("(g b p) d -> g p b d", p=128, b=GBLK)

    with TileCtx(nc) as tc, ExitStack() as ctx:
        consts = ctx.enter_context(tc.tile_pool(name="consts", bufs=1))
        io = ctx.enter_context(tc.tile_pool(name="io", bufs=3))
        work = ctx.enter_context(tc.tile_pool(name="work", bufs=2))
        saves = ctx.enter_context(tc.tile_pool(name="saves", bufs=2))
        stats = ctx.enter_context(tc.tile_pool(name="stats", bufs=2))
        zpf = [
            ctx.enter_context(tc.tile_pool(name=f"zpf{s}", bufs=2, space="PSUM"))
            for s in range(2 if GBLK <= 4 else 1)
        ]
        zpb = [
            ctx.enter_context(tc.tile_pool(name=f"zpb{s}", bufs=1, space="PSUM"))
            for s in range(2 if GBLK <= 4 else 1)
        ]
        tpool = ctx.enter_context(tc.tile_pool(name="tp", bufs=2, space="PSUM"))

        wfs = consts.tile([128, 6, 128], FP16, tag="wfs")
        wbs = consts.tile([128, 6, 128], FP16, tag="wbs")
        cbs = consts.tile([1, 6, 128], FP16, tag="cbs")
        ones1 = consts.tile([1, 128], FP16, tag="ones1")
        ident = consts.tile([128, 128], FP16, tag="ident")
        epsb = consts.tile([128, 1], F32, tag="epsb")
        nc.vector.memset(epsb, LN_EPS)
        nc.gpsimd.dma_start(out=wfs[:], in_=wf_in[:, :, :])
        nc.gpsimd.dma_start(out=wbs[:], in_=wb_in[:, :, :])
        nc.gpsimd.dma_start(out=cbs[:], in_=cb_in[:, :, :])
        nc.gpsimd.dma_start(out=ones1[:], in_=ones_in[:, :])
        nc.gpsimd.dma_start(out=ident[:], in_=id_in[:, :])

        def mm_layer(zp, Xst, widx, bias_idx, weights):
            for b in range(GBLK):
                if bias_idx is not None:
                    nc.tensor.matmul(
                        zp[:, b, :], ones1[:, :], cbs[:, bias_idx, :],
                        start=True, stop=False,
                    )
                nc.tensor.matmul(
                    zp[:, b, :], Xst[:, b, :], weights[:, widx, :],
                    start=(bias_idx is None), stop=True,
                )

        def transpose_to(src_h, tag):
            tp = tpool.tile([128, GBLK, 128], FP16, tag="tp")
            for b in range(GBLK):
                nc.tensor.transpose(tp[:, b, :], src_h[:, b, :], ident[:, :])
            dst = work.tile([128, GBLK, 128], FP16, tag=tag)
            _copy(nc, dst, tp)
            return dst

        def emit_fwd(g, s, out):
            """Generator: forward for group g on stream s; appends saves to out."""
            pf = io.tile([128, GBLK, 128], F32, tag=f"pin{s}")
            nc.sync.dma_start(out=pf, in_=pv[g])
            ph = work.tile([128, GBLK, 128], FP16, tag=f"ph{s}")
            nc.gpsimd.tensor_copy(ph, pf)
            Xst = transpose_to(ph, f"xstf{s}")
            yield

            for i in range(L):
                zp = zpf[s].tile([128, GBLK, 128], F32, tag=f"zpf{s}")
                mm_layer(zp, Xst, i, i, wfs)

                E = work.tile([128, GBLK, 128], F32, tag=f"E{s}")
                nc.scalar.activation(E, zp, AF.Exp)
                A = work.tile([128, GBLK, 128], F32, tag=f"A{s}")
                nc.scalar.activation(A, E, AF.Ln, bias=1.0)

                mu = stats.tile([128, GBLK, 1], F32, tag=f"mu{i}{s}")
                nc.vector.reduce_sum(out=mu, in_=A, axis=mybir.AxisListType.X)
                nc.gpsimd.tensor_scalar_mul(mu, mu, 1.0 / H)

                xc = saves.tile([128, GBLK, 128], FP16, tag=f"xc{i}{s}")
                nc.vector.tensor_tensor(
                    out=xc, in0=A, in1=_bview(mu, 128), op=OP.subtract
                )
                sqc = work.tile([128, GBLK, 128], FP16, tag=f"sqc{s}")
                nc.scalar.activation(sqc, xc, AF.Square)
                s2 = stats.tile([128, GBLK, 1], F32, tag=f"s2{s}")
                nc.vector.reduce_sum(out=s2, in_=sqc, axis=mybir.AxisListType.X)
                rstd = stats.tile([128, GBLK, 1], F32, tag=f"rstd{i}{s}")
                lnv = stats.tile([128, GBLK, 1], F32, tag=f"lnv{s}")
                nc.scalar.activation(lnv, s2, AF.Ln, bias=epsb[:, :], scale=1.0 / H)
                nc.scalar.activation(rstd, lnv, AF.Exp, scale=-0.5)

                xh = work.tile([128, GBLK, 128], FP16, tag=f"xh{s}")
                nc.vector.tensor_tensor(
                    out=xh, in0=xc, in1=_bview(rstd, 128), op=OP.mult
                )
                Xst = transpose_to(xh, f"xstf{s}")
                out.append((xc, rstd, mu))
                yield

            zp = zpf[s].tile([128, GBLK, 128], F32, tag=f"zpf{s}")
            mm_layer(zp, Xst, 4, 4, wfs)
            yo = io.tile([128, GBLK, 128], F32, tag=f"yout{s}")
            nc.scalar.copy(yo, zp)
            nc.sync.dma_start(out=dpv[g], in_=yo)
            yield

        def emit_bwd(g, s, sv):
            wf = io.tile([128, GBLK, 128], F32, tag=f"win{s}")
            nc.sync.dma_start(out=wf, in_=wv[g])
            wh = work.tile([128, GBLK, 128], FP16, tag=f"wh{s}")
            nc.gpsimd.tensor_copy(wh, wf)
            dz = wh
            yield

            for i in range(L - 1, -1, -1):
                Gst = transpose_to(dz, f"xstb{s}")
                gp = zpb[s].tile([128, GBLK, 128], F32, tag=f"zpb{s}")
                mm_layer(gp, Gst, 3 - i, None, wbs)  # gdy centered (mean folded)
                gd = work.tile([128, GBLK, 128], FP16, tag=f"gd{s}")
                nc.scalar.copy(gd, gp)

                xc, rstd, mu = sv[i]
                prod = work.tile([128, GBLK, 128], FP16, tag=f"prod{s}")
                m2 = stats.tile([128, GBLK, 1], F32, tag=f"m2{s}")
                for b in range(GBLK):
                    nc.vector.scalar_tensor_tensor(
                        out=prod[:, b, :], in0=xc[:, b, :], scalar=rstd[:, b, :],
                        in1=gd[:, b, :], op0=OP.mult, op1=OP.mult,
                        accum_out=m2[:, b, :],
                    )
                # q = rstd*m2/H so that xc*q = xhat*mean(xhat*gdy)
                q = stats.tile([128, GBLK, 1], F32, tag=f"q{s}")
                nc.gpsimd.tensor_tensor(out=q, in0=m2, in1=rstd, op=OP.mult)
                nc.gpsimd.tensor_scalar_mul(q, q, 1.0 / H)
                # dxn = xc*q - gd = xhat*mean(xhat*gdy) - gdy
                dxn = work.tile([128, GBLK, 128], FP16, tag=f"dxn{s}")
                for b in range(GBLK):
                    nc.vector.scalar_tensor_tensor(
                        out=dxn[:, b, :], in0=xc[:, b, :], scalar=q[:, b, :],
                        in1=gd[:, b, :], op0=OP.mult, op1=OP.subtract,
                    )
                # u = exp(-(xc+mu)) = 1-sig ; srn = rstd*u - rstd = -rstd*sig
                nmu = stats.tile([128, GBLK, 1], F32, tag=f"nmu{s}")
                nc.gpsimd.tensor_scalar_mul(nmu, mu, -1.0)
                emu = stats.tile([128, GBLK, 1], F32, tag=f"emu{s}")
                nc.scalar.activation(emu, nmu, AF.Exp)
                s1 = work.tile([128, GBLK, 128], FP16, tag=f"s1{s}")
                nc.scalar.activation(s1, xc, AF.Exp, scale=-1.0)
                er = stats.tile([128, GBLK, 1], F32, tag=f"er{s}")
                nc.gpsimd.tensor_tensor(out=er, in0=emu, in1=rstd, op=OP.mult)
                # srn = s1*(emu*rstd) - rstd = rstd*u - rstd = -rstd*sig
                srn = work.tile([128, GBLK, 128], FP16, tag=f"srn{s}")
                for b in range(GBLK):
                    nc.vector.tensor_scalar(
                        out=srn[:, b, :], in0=s1[:, b, :],
                        scalar1=er[:, b, :], scalar2=rstd[:, b, :],
                        op0=OP.mult, op1=OP.subtract,
                    )
                # dz = dxn*srn = (gdy - xhat*m2)*rstd*sig
                dz = work.tile([128, GBLK, 128], FP16, tag=f"dz{s}")
                nc.vector.tensor_tensor(out=dz, in0=dxn, in1=srn, op=OP.mult)
                yield

            Gst = transpose_to(dz, f"xstb{s}")
            gp = zpb[s].tile([128, GBLK, 128], F32, tag=f"zpb{s}")
            mm_layer(gp, Gst, 4, None, wbs)
            dwo = io.tile([128, GBLK, 128], F32, tag=f"dwout{s}")
            nc.scalar.copy(dwo, gp)
            nc.sync.dma_start(out=dwv[g], in_=dwo)
            yield

        def drive(gens):
            gens = [iter(x) for x in gens]
            while gens:
                nxt = []
                for it in gens:
                    try:
                        next(it)
                        nxt.append(it)
                    except StopIteration:
                        pass
                gens = nxt

        # rolling lockstep pipeline: bwd tails overlap the next pair's fwd head
        NS = 2 if GBLK <= 4 else 1
        assert NG % NS == 0
        live = []

        def drive_until(targets):
            while any(t in live for t in targets):
                for it in list(live):
                    try:
                        next(it)
                    except StopIteration:
                        live.remove(it)

        for k in range(NG // NS):
            svs = [[] for _ in range(NS)]
            fg = [emit_fwd(NS * k + s, s, svs[s]) for s in range(NS)]
            fg = [iter(x) for x in fg]
            live.extend(fg)
            drive_until(fg)
            live.extend(iter(x) for x in
                        [emit_bwd(NS * k + s, s, svs[s]) for s in range(NS)])
        drive_until(list(live))

# tile.TileContext import indirection (kept here so _emit reads cleanly)
TileCtx = tile.TileContext


def _host_precompute(t, W_in, b_in, fw, fb, gamma, beta, Wl, bl, W_out, b_out):
    t = np.asarray(t, dtype=np.float32).reshape(-1)[0]
    s = np.sin(t * np.asarray(fw, np.float32) + np.asarray(fb, np.float32))  # [L, H]
    Wl = np.asarray(Wl, np.float32)
    gamma = np.asarray(gamma, np.float32)
    beta = np.asarray(beta, np.float32)
    bl = np.asarray(bl, np.float32)
    W_in = np.asarray(W_in, np.float32)
    W_out = np.asarray(W_out, np.float32)
    b_in = np.asarray(b_in, np.float32)
    b_out = np.asarray(b_out, np.float32)

    Wg = [Wl[i] * gamma[i][None, :] for i in range(L)]          # [H, H]
    bg = [bl[i] + Wl[i] @ beta[i] for i in range(L)]            # [H]

    # fuse h4->y: y = xhat3 @ (W_out@Wg3).T + (b_out + W_out@bg3)
    M2 = (W_out.astype(np.float64) @ Wg[L - 1].astype(np.float64)).astype(np.float32)
    c = np.zeros((6, 128), np.float32)
    c[0] = b_in + s[0]
    for i in range(1, L):
        c[i] = bg[i - 1] + s[i]
    c[4] = b_out + W_out @ bg[L - 1]
    WF = np.stack(
        [W_in.T] + [Wg[i].T for i in range(L - 1)] + [M2.T, M2.T], axis=0
    )  # [6, K, N]; slot 4 = fused final matmul (slot 5 unused)
    Wc = [Wg[i] - Wg[i].mean(axis=1, keepdims=True) for i in range(L - 1)]
    M2n = -M2
    M2c = M2n - M2n.mean(axis=1, keepdims=True)
    WB = np.stack([M2c, Wc[2], Wc[1], Wc[0], W_in, W_in], axis=0)

    WF = np.ascontiguousarray(np.transpose(WF, (1, 0, 2))).astype(np.float16)
    WB = np.ascontiguousarray(np.transpose(WB, (1, 0, 2))).astype(np.float16)
    CB = c.astype(np.float16)[None, :, :]
    ONES = np.ones((1, 128), np.float16)
    EYE = np.eye(128, dtype=np.float16)
    return WF, WB, CB, ONES, EYE


_NC_CACHE = {}


def _get_nc(R, GBLK):
    key = (R, GBLK)
    if key not in _NC_CACHE:
        nc = bacc.Bacc("TRN2")
        _emit(nc, R, GBLK)
        nc.finalize()
        _NC_CACHE[key] = nc
    return _NC_CACHE[key]


def _run(p, w, consts, R, GBLK, n_cores):
    WF, WB, CB, ONES, EYE = consts
    nc = _get_nc(R, GBLK)
    in_maps = []
    for k in range(n_cores):
        in_maps.append(
            {
                "p": np.ascontiguousarray(p[k * R : (k + 1) * R]),
                "w": np.ascontiguousarray(w[k * R : (k + 1) * R]),
                "wf": WF,
                "wb": WB,
                "cb": CB,
                "ones1": ONES,
                "ident": EYE,
            }
        )
    res = run_bass_kernel_spmd(nc, in_maps, core_ids=list(range(n_cores)))
    dp = np.concatenate([r["dp"] for r in res.results], axis=0)
    dw = np.concatenate([r["dw"] for r in res.results], axis=0)
    return dp, dw


def kernel(t, p, w, W_in, b_in, fw, fb, gamma, beta, Wl, bl, W_out, b_out):
    consts = _host_precompute(
        t, W_in, b_in, fw, fb, gamma, beta, Wl, bl, W_out, b_out
    )
    p = np.asarray(p, np.float32)
    w = np.asarray(w, np.float32)
    R = p.shape[0] // NCORES
    dp, dw = _run(p, w, consts, R, GBLK=4, n_cores=NCORES)
    return dp, dw



# revision 35
# speedup vs baseline: 1.9397x; 1.9397x over previous
"""Trainium2 Bass kernel for nn_CLNF_54769422959177.

Computes (dp, dw) where dp = vf(p) (4-layer VectorField MLP forward) and
dw = -vjp(vf, p)(w), data-parallel over 8 NeuronCores.

v2 layout (vs the v1 baseline):
- A single manual InstLoadActFuncSet(natural_log_exp_and_others) at program
  start: every ACT func used (Exp/Ln/Copy) lives in that one table, so the
  finalize pass inserts no further table loads (v1 thrashed 443 loads
  = 568us on the ACT engine).
- LN stats via one big bn_stats + per-block bn_aggr (mean+var in one DVE
  pass) instead of Square + two reduces.
- Backward in xhat-form: dz = (gd - xhat*m)*rstd*sigmoid, with m from a
  fused tensor_tensor_reduce (scale=1/H) and sigmoid = 1 - exp(-A) from the
  saved softplus output A.
- fp16 tensor_scalar ops (4x DVE mode) for xhat / (u-1); per-block scalars
  come from bn_aggr / rstd tiles.
- f32 entry: p and w are PE-transposed and matmul'ed in f32 directly (PE has
  slack), skipping the f32->fp16 cast pass of v1.
- Engine balance: ACT = exp/ln/copies, DVE = bn/xhat/ttr/tt/PSUM-copies,
  Pool = gd copy + per-block stt chains.
"""

import numpy as np

import concourse.bass as bass
from concourse import bacc
import concourse.tile as tile
from concourse import mybir
from concourse.bass_utils import run_bass_kernel_spmd

B, D, H, L = 131072, 128, 128, 4
NCORES = 8
LN_EPS = 1e-5
FP16 = mybir.dt.float16
F32 = mybir.dt.float32
AF = mybir.ActivationFunctionType
OP = mybir.AluOpType
ACT_TABLE_NL_EXP = 6  # natural_log_exp_and_others in cayman act_info.json

TileCtx = tile.TileContext


def _emit(nc, R, GBLK):
    """Emit the per-core program: R rows, blocks of 128 rows, GBLK blocks/group."""
    NG = R // (GBLK * 128)
    assert NG * GBLK * 128 == R

    p_in = nc.dram_tensor("p", [R, D], F32, kind="ExternalInput")
    w_in = nc.dram_tensor("w", [R, D], F32, kind="ExternalInput")
    # moving weights fwd: [K, 5, N] = {W_in.T, Wg0.T, Wg1.T, Wg2.T, M2.T}
    wf_in = nc.dram_tensor("wf", [128, 5, 128], FP16, kind="ExternalInput")
    wf0_in = nc.dram_tensor("wf0", [128, 128], F32, kind="ExternalInput")
    # moving weights bwd: {M2c, Wc2, Wc1, Wc0, W_in}
    wb_in = nc.dram_tensor("wb", [128, 5, 128], FP16, kind="ExternalInput")
    wb0_in = nc.dram_tensor("wb0", [128, 128], F32, kind="ExternalInput")
    cb_in = nc.dram_tensor("cb", [1, 5, GBLK * 128], FP16, kind="ExternalInput")
    cb0_in = nc.dram_tensor("cb0", [1, GBLK * 128], F32, kind="ExternalInput")
    ones_in = nc.dram_tensor("ones1", [1, 128], FP16, kind="ExternalInput")
    ones32_in = nc.dram_tensor("ones1_32", [1, 128], F32, kind="ExternalInput")
    id_in = nc.dram_tensor("ident", [128, 128], FP16, kind="ExternalInput")
    id32_in = nc.dram_tensor("ident32", [128, 128], F32, kind="ExternalInput")
    dp_out = nc.dram_tensor("dp", [R, D], F32, kind="ExternalOutput")
    dw_out = nc.dram_tensor("dw", [R, D], F32, kind="ExternalOutput")

    pv = p_in[:, :].rearrange("(g b p) d -> g p b d", p=128, b=GBLK)
    wv = w_in[:, :].rearrange("(g b p) d -> g p b d", p=128, b=GBLK)
    dpv = dp_out[:, :].rearrange("(g b p) d -> g p b d", p=128, b=GBLK)
    dwv = dw_out[:, :].rearrange("(g b p) d -> g p b d", p=128, b=GBLK)

    from contextlib import ExitStack

    with TileCtx(nc) as tc, ExitStack() as ctx:
        NS = 4
        consts = ctx.enter_context(tc.tile_pool(name="consts", bufs=1))
        io = ctx.enter_context(tc.tile_pool(name="io", bufs=2))
        work = ctx.enter_context(tc.tile_pool(name="work", bufs=1))
        saves = ctx.enter_context(tc.tile_pool(name="saves", bufs=2))
        stats = ctx.enter_context(tc.tile_pool(name="stats", bufs=2))
        # One PSUM bank per stream per direction; entry transposes, layer
        # matmuls and activation transposes all rotate through the same
        # single-buffer ring (their uses are chain-serial within a stream).
        zpf = [
            ctx.enter_context(tc.tile_pool(name=f"zpf{s}", bufs=1, space="PSUM"))
            for s in range(NS)
        ]
        zpb = [
            ctx.enter_context(tc.tile_pool(name=f"zpb{s}", bufs=1, space="PSUM"))
            for s in range(NS)
        ]

        wfs = consts.tile([128, 5, 128], FP16, tag="wfs")
        wf0 = consts.tile([128, 128], F32, tag="wf0")
        wbs = consts.tile([128, 5, 128], FP16, tag="wbs")
        wb0 = consts.tile([128, 128], F32, tag="wb0")
        cbs = consts.tile([1, 5, GBLK * 128], FP16, tag="cbs")
        cb0 = consts.tile([1, GBLK * 128], F32, tag="cb0")
        ones1 = consts.tile([1, 128], FP16, tag="ones1")
        ones1_32 = consts.tile([1, 128], F32, tag="ones1_32")
        ident = consts.tile([128, 128], FP16, tag="ident")
        ident32 = consts.tile([128, 128], F32, tag="ident32")
        epsb = consts.tile([128, 1], F32, tag="epsb")
        nc.vector.memset(epsb, LN_EPS)
        nc.gpsimd.dma_start(out=wfs[:], in_=wf_in[:, :, :])
        nc.gpsimd.dma_start(out=wf0[:], in_=wf0_in[:, :])
        nc.gpsimd.dma_start(out=wbs[:], in_=wb_in[:, :, :])
        nc.gpsimd.dma_start(out=wb0[:], in_=wb0_in[:, :])
        nc.gpsimd.dma_start(out=cbs[:], in_=cb_in[:, :, :])
        nc.gpsimd.dma_start(out=cb0[:], in_=cb0_in[:, :])
        nc.gpsimd.dma_start(out=ones1[:], in_=ones_in[:, :])
        nc.gpsimd.dma_start(out=ones1_32[:], in_=ones32_in[:, :])
        nc.gpsimd.dma_start(out=ident[:], in_=id_in[:, :])
        nc.gpsimd.dma_start(out=ident32[:], in_=id32_in[:, :])

        # One activation table covering Exp, Ln, Copy: loaded once, the
        # finalize fixpoint then inserts no per-activation loads.
        ld = mybir.InstLoadActFuncSet(
            name=nc.get_next_instruction_name(), ins=[], outs=[]
        )
        ld.act_func_set_id = ACT_TABLE_NL_EXP
        nc.scalar.add_instruction(ld)

        def emit_fwd(g, s, out):
            """Generator: forward for group g on stream s; appends saves."""
            pf = io.tile([128, GBLK, 128], F32, tag=f"pin{s}")
            nc.sync.dma_start(out=pf, in_=pv[g])
            # entry: f32 transpose + f32 copy to SBUF (no fp16 cast pass)
            tpe = zpf[s].tile([128, GBLK, 128], F32, tag=f"zpf{s}")
            for b in range(GBLK):
                nc.tensor.transpose(tpe[:, b, :], pf[:, b, :], ident32[:, :])
            Xst32 = work.tile([128, GBLK, 128], F32, tag=f"xst32{s}")
            nc.scalar.copy(Xst32, tpe)
            yield

            Xst16 = None
            for i in range(L):
                zp = zpf[s].tile([128, GBLK, 128], F32, tag=f"zpf{s}")
                if i == 0:
                    nc.tensor.matmul(
                        zp[:, :, :], ones1_32[:, :], cb0[:, :],
                        start=True, stop=False, skip_group_check=True,
                    )
                    for b in range(GBLK):
                        nc.tensor.matmul(
                            zp[:, b, :], Xst32[:, b, :], wf0[:, :],
                            start=False, stop=True, skip_group_check=True,
                        )
                else:
                    nc.tensor.matmul(
                        zp[:, :, :], ones1[:, :], cbs[:, i, :],
                        start=True, stop=False, skip_group_check=True,
                    )
                    for b in range(GBLK):
                        nc.tensor.matmul(
                            zp[:, b, :], Xst16[:, b, :], wfs[:, i, :],
                            start=False, stop=True, skip_group_check=True,
                        )

                E = work.tile([128, GBLK, 128], F32, tag=f"E{s}")
                nc.scalar.activation(E, zp, AF.Exp)
                A16 = saves.tile([128, GBLK, 128], FP16, tag=f"A{i}{s}")
                nc.scalar.activation(A16, E, AF.Ln, bias=1.0)
                yield

                st6 = stats.tile([128, GBLK, 6], F32, tag=f"st6{s}")
                aggr = saves.tile([128, GBLK, 2], F32, tag=f"ag{i}{s}")
                for b in range(2):
                    nc.vector.bn_stats(out=st6[:, b, :], in_=A16[:, b, :])
                yield
                for b in range(2, GBLK):
                    nc.vector.bn_stats(out=st6[:, b, :], in_=A16[:, b, :])
                for b in range(2):
                    nc.vector.bn_aggr(out=aggr[:, b, :], in_=st6[:, b, :])
                yield
                for b in range(2, GBLK):
                    nc.vector.bn_aggr(out=aggr[:, b, :], in_=st6[:, b, :])
                lnv = stats.tile([128, GBLK, 1], F32, tag=f"lnv{s}")
                nc.scalar.activation(
                    lnv, aggr[:, :, 1:2], AF.Ln, bias=epsb[:, :]
                )
                rstd = saves.tile([128, GBLK, 1], F32, tag=f"rs{i}{s}")
                nc.scalar.activation(rstd, lnv, AF.Exp, scale=-0.5)
                yield

                xh16 = saves.tile([128, GBLK, 128], FP16, tag=f"xh{i}{s}")
                for b in range(2):
                    nc.vector.tensor_scalar(
                        out=xh16[:, b, :], in0=A16[:, b, :],
                        scalar1=aggr[:, b, 0:1], scalar2=rstd[:, b, :],
                        op0=OP.subtract, op1=OP.mult,
                    )
                yield
                tp = zpf[s].tile([128, GBLK, 128], FP16, tag=f"zpf{s}")
                for b in range(2, GBLK):
                    nc.vector.tensor_scalar(
                        out=xh16[:, b, :], in0=A16[:, b, :],
                        scalar1=aggr[:, b, 0:1], scalar2=rstd[:, b, :],
                        op0=OP.subtract, op1=OP.mult,
                    )
                for b in range(2):
                    nc.tensor.transpose(tp[:, b, :], xh16[:, b, :], ident[:, :])
                yield
                for b in range(2, GBLK):
                    nc.tensor.transpose(tp[:, b, :], xh16[:, b, :], ident[:, :])
                Xst16 = work.tile([128, GBLK, 128], FP16, tag=f"xst{s}")
                nc.vector.tensor_scalar(
                    out=Xst16, in0=tp, scalar1=1.0, scalar2=None, op0=OP.mult
                )
                out.append((A16, aggr, rstd, xh16))
                yield

            zp = zpf[s].tile([128, GBLK, 128], F32, tag=f"zpf{s}")
            nc.tensor.matmul(
                zp[:, :, :], ones1[:, :], cbs[:, 4, :],
                start=True, stop=False, skip_group_check=True,
            )
            for b in range(GBLK):
                nc.tensor.matmul(
                    zp[:, b, :], Xst16[:, b, :], wfs[:, 4, :],
                    start=False, stop=True, skip_group_check=True,
                )
            yo = io.tile([128, GBLK, 128], F32, tag=f"yout{s}")
            nc.scalar.copy(yo, zp)
            # issue the store from ACT (the producer) so the DMA wait is
            # satisfied by construction and never parks the SP sequencer
            nc.scalar.dma_start(out=dpv[g], in_=yo)
            yield

        def emit_bwd(g, s, sv):
            wf = io.tile([128, GBLK, 128], F32, tag=f"win{s}")
            nc.sync.dma_start(out=wf, in_=wv[g])
            tpe = zpb[s].tile([128, GBLK, 128], F32, tag=f"zpb{s}")
            for b in range(GBLK):
                nc.tensor.transpose(tpe[:, b, :], wf[:, b, :], ident32[:, :])
            Gst32 = work.tile([128, GBLK, 128], F32, tag=f"gst32{s}")
            nc.vector.tensor_scalar(
                out=Gst32, in0=tpe, scalar1=1.0, scalar2=None, op0=OP.mult
            )
            yield

            Gst16 = None
            for i in range(L - 1, -1, -1):
                A16, aggr, rstd, xh16 = sv[i]
                # u = exp(-A) = 1 - sigmoid of the pre-softplus input;
                # independent of the matmul chain, issue early.
                u16 = work.tile([128, GBLK, 128], FP16, tag=f"u{s}")
                nc.scalar.activation(u16, A16, AF.Exp, scale=-1.0)

                gp = zpb[s].tile([128, GBLK, 128], F32, tag=f"zpb{s}")
                if i == L - 1:
                    for b in range(GBLK):
                        nc.tensor.matmul(
                            gp[:, b, :], Gst32[:, b, :], wb0[:, :],
                            start=True, stop=True,
                        )
                else:
                    for b in range(GBLK):
                        nc.tensor.matmul(
                            gp[:, b, :], Gst16[:, b, :], wbs[:, 3 - i, :],
                            start=True, stop=True,
                        )
                gd16 = work.tile([128, GBLK, 128], FP16, tag=f"gd{s}")
                nc.scalar.copy(gd16, gp)
                yield

                # srn = rstd*u - rstd (indep of gd) interleaved with the
                # m2 = sum(xhat*gd) reduction (indep of u) so the DVE wait
                # queue never fills with one not-ready dependency group.
                srn = work.tile([128, GBLK, 128], FP16, tag=f"srn{s}")
                pr = work.tile([128, GBLK, 128], FP16, tag=f"pr{s}")
                m2 = stats.tile([128, GBLK, 1], F32, tag=f"m2{s}")
                for b in range(2):
                    nc.vector.tensor_scalar(
                        out=srn[:, b, :], in0=u16[:, b, :],
                        scalar1=rstd[:, b, :], scalar2=rstd[:, b, :],
                        op0=OP.mult, op1=OP.subtract,
                    )
                    nc.vector.scalar_tensor_tensor(
                        out=pr[:, b, :], in0=xh16[:, b, :], scalar=1.0,
                        in1=gd16[:, b, :], op0=OP.mult, op1=OP.mult,
                        accum_out=m2[:, b, :],
                    )
                yield
                for b in range(2, GBLK):
                    nc.vector.tensor_scalar(
                        out=srn[:, b, :], in0=u16[:, b, :],
                        scalar1=rstd[:, b, :], scalar2=rstd[:, b, :],
                        op0=OP.mult, op1=OP.subtract,
                    )
                    nc.vector.scalar_tensor_tensor(
                        out=pr[:, b, :], in0=xh16[:, b, :], scalar=1.0,
                        in1=gd16[:, b, :], op0=OP.mult, op1=OP.mult,
                        accum_out=m2[:, b, :],
                    )
                q = stats.tile([128, GBLK, 1], F32, tag=f"q{s}")
                nc.gpsimd.tensor_scalar_mul(q, m2, 1.0 / H)
                yield

                # xm = xhat*q (4x tensor_scalar), dxn = xm - gd
                xm = work.tile([128, GBLK, 128], FP16, tag=f"xm{s}")
                for b in range(2):
                    nc.vector.tensor_scalar(
                        out=xm[:, b, :], in0=xh16[:, b, :],
                        scalar1=q[:, b, :], scalar2=None, op0=OP.mult,
                    )
                yield
                for b in range(2, GBLK):
                    nc.vector.tensor_scalar(
                        out=xm[:, b, :], in0=xh16[:, b, :],
                        scalar1=q[:, b, :], scalar2=None, op0=OP.mult,
                    )
                dxn = work.tile([128, GBLK, 128], FP16, tag=f"dxn{s}")
                nc.vector.tensor_tensor(
                    out=dxn, in0=xm, in1=gd16, op=OP.subtract
                )
                yield
                # dz = dxn * srn = (gd - xhat*m) * rstd * sigmoid
                dz = work.tile([128, GBLK, 128], FP16, tag=f"dz{s}")
                nc.gpsimd.tensor_tensor(out=dz, in0=dxn, in1=srn, op=OP.mult)
                yield
                tp = zpb[s].tile([128, GBLK, 128], FP16, tag=f"zpb{s}")
                for b in range(2):
                    nc.tensor.transpose(tp[:, b, :], dz[:, b, :], ident[:, :])
                yield
                for b in range(2, GBLK):
                    nc.tensor.transpose(tp[:, b, :], dz[:, b, :], ident[:, :])
                Gst16 = work.tile([128, GBLK, 128], FP16, tag=f"gst{s}")
                nc.vector.tensor_scalar(
                    out=Gst16, in0=tp, scalar1=1.0, scalar2=None, op0=OP.mult
                )
                yield

            gp = zpb[s].tile([128, GBLK, 128], F32, tag=f"zpb{s}")
            for b in range(GBLK):
                nc.tensor.matmul(
                    gp[:, b, :], Gst16[:, b, :], wbs[:, 4, :],
                    start=True, stop=True,
                )
            dwo = io.tile([128, GBLK, 128], F32, tag=f"dwout{s}")
            nc.scalar.copy(dwo, gp)
            nc.scalar.dma_start(out=dwv[g], in_=dwo)
            yield

        # Free-running phase-staggered pipeline: each stream s processes
        # groups s, s+NS, ... as one continuous fwd->bwd chain; streams are
        # primed with an emission-offset so their phases stay staggered and
        # every engine always sees ready work from some stream.
        assert NG % NS == 0

        def stream_gen(s):
            for g in range(s, NG, NS):
                sv = []
                yield from emit_fwd(g, s, sv)
                yield from emit_bwd(g, s, sv)

        gens = [iter(stream_gen(s)) for s in range(NS)]
        live = []
        PRIME = 20  # chunks of head-start between adjacent streams
        for s in range(NS):
            live.append(gens[s])
            for it in list(live):
                for _ in range(PRIME if it is gens[s] else 1):
                    try:
                        next(it)
                    except StopIteration:
                        if it in live:
                            live.remove(it)
                        break
        while live:
            for it in list(live):
                try:
                    next(it)
                except StopIteration:
                    live.remove(it)


def _host_precompute(t, W_in, b_in, fw, fb, gamma, beta, Wl, bl, W_out, b_out):
    t = np.asarray(t, dtype=np.float32).reshape(-1)[0]
    s = np.sin(t * np.asarray(fw, np.float32) + np.asarray(fb, np.float32))  # [L, H]
    Wl = np.asarray(Wl, np.float32)
    gamma = np.asarray(gamma, np.float32)
    beta = np.asarray(beta, np.float32)
    bl = np.asarray(bl, np.float32)
    W_in = np.asarray(W_in, np.float32)
    W_out = np.asarray(W_out, np.float32)
    b_in = np.asarray(b_in, np.float32)
    b_out = np.asarray(b_out, np.float32)

    Wg = [Wl[i] * gamma[i][None, :] for i in range(L)]          # [H, H]
    bg = [bl[i] + Wl[i] @ beta[i] for i in range(L)]            # [H]

    # fuse h4->y: y = xhat3 @ (W_out@Wg3).T + (b_out + W_out@bg3)
    M2 = (W_out.astype(np.float64) @ Wg[L - 1].astype(np.float64)).astype(np.float32)
    c = np.zeros((5, 128), np.float32)
    c[0] = b_in + s[0]
    for i in range(1, L):
        c[i] = bg[i - 1] + s[i]
    c[4] = b_out + W_out @ bg[L - 1]
    WF = np.stack(
        [W_in.T] + [Wg[i].T for i in range(L - 1)] + [M2.T], axis=0
    )  # [5, K, N]
    Wc = [Wg[i] - Wg[i].mean(axis=1, keepdims=True) for i in range(L - 1)]
    M2n = -M2
    M2c = M2n - M2n.mean(axis=1, keepdims=True)
    WB = np.stack([M2c, Wc[2], Wc[1], Wc[0], W_in], axis=0)

    WF16 = np.ascontiguousarray(np.transpose(WF, (1, 0, 2))).astype(np.float16)
    WF0 = np.ascontiguousarray(W_in.T)  # [K, N] f32
    WB16 = np.ascontiguousarray(np.transpose(WB, (1, 0, 2))).astype(np.float16)
    WB0 = np.ascontiguousarray(M2c)  # f32
    GBLK = 4
    CB = np.tile(c, (1, GBLK)).astype(np.float16)[None, :, :]  # [1, 5, GBLK*128]
    CB0 = np.tile(c[0:1, :], (1, GBLK)).astype(np.float32)     # [1, GBLK*128]
    ONES = np.ones((1, 128), np.float16)
    ONES32 = np.ones((1, 128), np.float32)
    EYE = np.eye(128, dtype=np.float16)
    EYE32 = np.eye(128, dtype=np.float32)
    return WF16, WF0, WB16, WB0, CB, CB0, ONES, ONES32, EYE, EYE32


_NC_CACHE = {}


def _get_nc(R, GBLK):
    key = (R, GBLK)
    if key not in _NC_CACHE:
        nc = bacc.Bacc("TRN2")
        _emit(nc, R, GBLK)
        nc.finalize()
        _NC_CACHE[key] = nc
    return _NC_CACHE[key]


def _run(p, w, consts, R, GBLK, n_cores):
    WF16, WF0, WB16, WB0, CB, CB0, ONES, ONES32, EYE, EYE32 = consts
    nc = _get_nc(R, GBLK)
    in_maps = []
    for k in range(n_cores):
        in_maps.append(
            {
                "p": np.ascontiguousarray(p[k * R : (k + 1) * R]),
                "w": np.ascontiguousarray(w[k * R : (k + 1) * R]),
                "wf": WF16,
                "wf0": WF0,
                "wb": WB16,
                "wb0": WB0,
                "cb": CB,
                "cb0": CB0,
                "ones1": ONES,
                "ones1_32": ONES32,
                "ident": EYE,
                "ident32": EYE32,
            }
        )
    res = run_bass_kernel_spmd(nc, in_maps, core_ids=list(range(n_cores)))
    dp = np.concatenate([r["dp"] for r in res.results], axis=0)
    dw = np.concatenate([r["dw"] for r in res.results], axis=0)
    return dp, dw


def kernel(t, p, w, W_in, b_in, fw, fb, gamma, beta, Wl, bl, W_out, b_out):
    consts = _host_precompute(
        t, W_in, b_in, fw, fb, gamma, beta, Wl, bl, W_out, b_out
    )
    p = np.asarray(p, np.float32)
    w = np.asarray(w, np.float32)
    R = p.shape[0] // NCORES
    dp, dw = _run(p, w, consts, R, GBLK=4, n_cores=NCORES)
    return dp, dw


# revision 37
# speedup vs baseline: 1.9425x; 1.0014x over previous
"""Trainium2 Bass kernel for nn_CLNF_54769422959177.

Computes (dp, dw) where dp = vf(p) (4-layer VectorField MLP forward) and
dw = -vjp(vf, p)(w), data-parallel over 8 NeuronCores.

v3 design (1178909 -> 606915 ns vs the v1 baseline):
- A single manual InstLoadActFuncSet(natural_log_exp_and_others) at program
  start: every ACT func used (Exp/Ln/Copy) lives in that one table, so the
  finalize pass inserts no further table loads (v1 thrashed 443 loads
  = 568us on the ACT engine).
- LN stats via per-block bn_stats/bn_aggr (mean+var in one DVE pass)
  instead of Square + two reduces; rstd = exp(-0.5*ln(var+eps)).
- Backward in xhat-form: dz = (gd - xhat*m)*rstd*sigmoid, with m2 from
  scalar_tensor_tensor+accum and sigmoid = 1 - exp(-A) from the saved
  softplus output A (no recompute of the forward pre-activation).
- fp16 tensor_scalar ops (4x DVE mode) for xhat/srn/xm with per-block
  [128,1] stat scalars; engine split: ACT = exp/ln + PSUM evacuations,
  DVE = bn/stt/ts/tt + transpose copies, Pool = dz + small stat ops.
- f32 entry: p and w are PE-transposed and matmul'ed in f32 directly (PE
  has slack), skipping the f32->fp16 cast pass of v1.
- NS=4 phase-staggered free-running streams; one PSUM bank per stream per
  direction (entry transposes, matmuls and activation transposes share a
  single-buffer ring - all chain-serial within a stream); batched bias
  matmul (one wide K=1 matmul, no ones<->Xst ldweights ping-pong); per-block
  instruction quads split/interleaved across yields so the 4-deep in-order
  engine wait queues don't head-of-line block on one late dependency;
  output-store DMAs issued from the producing engine (ACT) so the SP
  sequencer never parks on result availability.
"""

import numpy as np

import concourse.bass as bass
from concourse import bacc
import concourse.tile as tile
from concourse import mybir
from concourse.bass_utils import run_bass_kernel_spmd

B, D, H, L = 131072, 128, 128, 4
NCORES = 8
LN_EPS = 1e-5
FP16 = mybir.dt.float16
F32 = mybir.dt.float32
AF = mybir.ActivationFunctionType
OP = mybir.AluOpType
ACT_TABLE_NL_EXP = 6  # natural_log_exp_and_others in cayman act_info.json

TileCtx = tile.TileContext


def _emit(nc, R, GBLK):
    """Emit the per-core program: R rows, blocks of 128 rows, GBLK blocks/group."""
    NG = R // (GBLK * 128)
    assert NG * GBLK * 128 == R

    p_in = nc.dram_tensor("p", [R, D], F32, kind="ExternalInput")
    w_in = nc.dram_tensor("w", [R, D], F32, kind="ExternalInput")
    # moving weights fwd: [K, 5, N] = {W_in.T, Wg0.T, Wg1.T, Wg2.T, M2.T}
    wf_in = nc.dram_tensor("wf", [128, 5, 128], FP16, kind="ExternalInput")
    wf0_in = nc.dram_tensor("wf0", [128, 128], F32, kind="ExternalInput")
    # moving weights bwd: {M2c, Wc2, Wc1, Wc0, W_in}
    wb_in = nc.dram_tensor("wb", [128, 5, 128], FP16, kind="ExternalInput")
    wb0_in = nc.dram_tensor("wb0", [128, 128], F32, kind="ExternalInput")
    cb_in = nc.dram_tensor("cb", [1, 5, GBLK * 128], FP16, kind="ExternalInput")
    cb0_in = nc.dram_tensor("cb0", [1, GBLK * 128], F32, kind="ExternalInput")
    ones_in = nc.dram_tensor("ones1", [1, 128], FP16, kind="ExternalInput")
    ones32_in = nc.dram_tensor("ones1_32", [1, 128], F32, kind="ExternalInput")
    id_in = nc.dram_tensor("ident", [128, 128], FP16, kind="ExternalInput")
    id32_in = nc.dram_tensor("ident32", [128, 128], F32, kind="ExternalInput")
    dp_out = nc.dram_tensor("dp", [R, D], F32, kind="ExternalOutput")
    dw_out = nc.dram_tensor("dw", [R, D], F32, kind="ExternalOutput")

    pv = p_in[:, :].rearrange("(g b p) d -> g p b d", p=128, b=GBLK)
    wv = w_in[:, :].rearrange("(g b p) d -> g p b d", p=128, b=GBLK)
    dpv = dp_out[:, :].rearrange("(g b p) d -> g p b d", p=128, b=GBLK)
    dwv = dw_out[:, :].rearrange("(g b p) d -> g p b d", p=128, b=GBLK)

    from contextlib import ExitStack

    with TileCtx(nc) as tc, ExitStack() as ctx:
        NS = 4
        consts = ctx.enter_context(tc.tile_pool(name="consts", bufs=1))
        io = ctx.enter_context(tc.tile_pool(name="io", bufs=2))
        work = ctx.enter_context(tc.tile_pool(name="work", bufs=1))
        saves = ctx.enter_context(tc.tile_pool(name="saves", bufs=2))
        stats = ctx.enter_context(tc.tile_pool(name="stats", bufs=2))
        # One PSUM bank per stream per direction; entry transposes, layer
        # matmuls and activation transposes all rotate through the same
        # single-buffer ring (their uses are chain-serial within a stream).
        zpf = [
            ctx.enter_context(tc.tile_pool(name=f"zpf{s}", bufs=1, space="PSUM"))
            for s in range(NS)
        ]
        zpb = [
            ctx.enter_context(tc.tile_pool(name=f"zpb{s}", bufs=1, space="PSUM"))
            for s in range(NS)
        ]

        wfs = consts.tile([128, 5, 128], FP16, tag="wfs")
        wf0 = consts.tile([128, 128], F32, tag="wf0")
        wbs = consts.tile([128, 5, 128], FP16, tag="wbs")
        wb0 = consts.tile([128, 128], F32, tag="wb0")
        cbs = consts.tile([1, 5, GBLK * 128], FP16, tag="cbs")
        cb0 = consts.tile([1, GBLK * 128], F32, tag="cb0")
        ones1 = consts.tile([1, 128], FP16, tag="ones1")
        ones1_32 = consts.tile([1, 128], F32, tag="ones1_32")
        ident = consts.tile([128, 128], FP16, tag="ident")
        ident32 = consts.tile([128, 128], F32, tag="ident32")
        epsb = consts.tile([128, 1], F32, tag="epsb")
        nc.vector.memset(epsb, LN_EPS)
        nc.gpsimd.dma_start(out=wfs[:], in_=wf_in[:, :, :])
        nc.gpsimd.dma_start(out=wf0[:], in_=wf0_in[:, :])
        nc.gpsimd.dma_start(out=wbs[:], in_=wb_in[:, :, :])
        nc.gpsimd.dma_start(out=wb0[:], in_=wb0_in[:, :])
        nc.gpsimd.dma_start(out=cbs[:], in_=cb_in[:, :, :])
        nc.gpsimd.dma_start(out=cb0[:], in_=cb0_in[:, :])
        nc.gpsimd.dma_start(out=ones1[:], in_=ones_in[:, :])
        nc.gpsimd.dma_start(out=ones1_32[:], in_=ones32_in[:, :])
        nc.gpsimd.dma_start(out=ident[:], in_=id_in[:, :])
        nc.gpsimd.dma_start(out=ident32[:], in_=id32_in[:, :])

        # One activation table covering Exp, Ln, Copy: loaded once, the
        # finalize fixpoint then inserts no per-activation loads.
        ld = mybir.InstLoadActFuncSet(
            name=nc.get_next_instruction_name(), ins=[], outs=[]
        )
        ld.act_func_set_id = ACT_TABLE_NL_EXP
        nc.scalar.add_instruction(ld)

        def emit_fwd(g, s, out):
            """Generator: forward for group g on stream s; appends saves."""
            pf = io.tile([128, GBLK, 128], F32, tag=f"pin{s}")
            nc.sync.dma_start(out=pf, in_=pv[g])
            # entry: f32 transpose + f32 copy to SBUF (no fp16 cast pass)
            tpe = zpf[s].tile([128, GBLK, 128], F32, tag=f"zpf{s}")
            for b in range(GBLK):
                nc.tensor.transpose(tpe[:, b, :], pf[:, b, :], ident32[:, :])
            Xst32 = work.tile([128, GBLK, 128], F32, tag=f"xst32{s}")
            nc.scalar.copy(Xst32, tpe)
            yield

            Xst16 = None
            for i in range(L):
                zp = zpf[s].tile([128, GBLK, 128], F32, tag=f"zpf{s}")
                if i == 0:
                    nc.tensor.matmul(
                        zp[:, :, :], ones1_32[:, :], cb0[:, :],
                        start=True, stop=False, skip_group_check=True,
                    )
                    for b in range(GBLK):
                        nc.tensor.matmul(
                            zp[:, b, :], Xst32[:, b, :], wf0[:, :],
                            start=False, stop=True, skip_group_check=True,
                        )
                else:
                    nc.tensor.matmul(
                        zp[:, :, :], ones1[:, :], cbs[:, i, :],
                        start=True, stop=False, skip_group_check=True,
                    )
                    for b in range(GBLK):
                        nc.tensor.matmul(
                            zp[:, b, :], Xst16[:, b, :], wfs[:, i, :],
                            start=False, stop=True, skip_group_check=True,
                        )

                E = work.tile([128, GBLK, 128], F32, tag=f"E{s}")
                nc.scalar.activation(E, zp, AF.Exp)
                A16 = saves.tile([128, GBLK, 128], FP16, tag=f"A{i}{s}")
                nc.scalar.activation(A16, E, AF.Ln, bias=1.0)
                yield

                st6 = stats.tile([128, GBLK, 6], F32, tag=f"st6{s}")
                aggr = saves.tile([128, GBLK, 2], F32, tag=f"ag{i}{s}")
                for b in range(2):
                    nc.vector.bn_stats(out=st6[:, b, :], in_=A16[:, b, :])
                yield
                for b in range(2, GBLK):
                    nc.vector.bn_stats(out=st6[:, b, :], in_=A16[:, b, :])
                for b in range(2):
                    nc.vector.bn_aggr(out=aggr[:, b, :], in_=st6[:, b, :])
                yield
                for b in range(2, GBLK):
                    nc.vector.bn_aggr(out=aggr[:, b, :], in_=st6[:, b, :])
                lnv = stats.tile([128, GBLK, 1], F32, tag=f"lnv{s}")
                nc.scalar.activation(
                    lnv, aggr[:, :, 1:2], AF.Ln, bias=epsb[:, :]
                )
                rstd = saves.tile([128, GBLK, 1], F32, tag=f"rs{i}{s}")
                nc.scalar.activation(rstd, lnv, AF.Exp, scale=-0.5)
                yield

                xh16 = saves.tile([128, GBLK, 128], FP16, tag=f"xh{i}{s}")
                for b in range(2):
                    nc.vector.tensor_scalar(
                        out=xh16[:, b, :], in0=A16[:, b, :],
                        scalar1=aggr[:, b, 0:1], scalar2=rstd[:, b, :],
                        op0=OP.subtract, op1=OP.mult,
                    )
                yield
                tp = zpf[s].tile([128, GBLK, 128], FP16, tag=f"zpf{s}")
                for b in range(2, GBLK):
                    nc.vector.tensor_scalar(
                        out=xh16[:, b, :], in0=A16[:, b, :],
                        scalar1=aggr[:, b, 0:1], scalar2=rstd[:, b, :],
                        op0=OP.subtract, op1=OP.mult,
                    )
                for b in range(2):
                    nc.tensor.transpose(tp[:, b, :], xh16[:, b, :], ident[:, :])
                yield
                for b in range(2, GBLK):
                    nc.tensor.transpose(tp[:, b, :], xh16[:, b, :], ident[:, :])
                Xst16 = work.tile([128, GBLK, 128], FP16, tag=f"xst{s}")
                nc.vector.tensor_scalar(
                    out=Xst16, in0=tp, scalar1=1.0, scalar2=None, op0=OP.mult
                )
                out.append((A16, aggr, rstd, xh16))
                yield

            zp = zpf[s].tile([128, GBLK, 128], F32, tag=f"zpf{s}")
            nc.tensor.matmul(
                zp[:, :, :], ones1[:, :], cbs[:, 4, :],
                start=True, stop=False, skip_group_check=True,
            )
            for b in range(GBLK):
                nc.tensor.matmul(
                    zp[:, b, :], Xst16[:, b, :], wfs[:, 4, :],
                    start=False, stop=True, skip_group_check=True,
                )
            yo = io.tile([128, GBLK, 128], F32, tag=f"yout{s}")
            nc.scalar.copy(yo, zp)
            # issue the store from ACT (the producer) so the DMA wait is
            # satisfied by construction and never parks the SP sequencer
            nc.scalar.dma_start(out=dpv[g], in_=yo)
            yield

        def emit_bwd(g, s, sv):
            wf = io.tile([128, GBLK, 128], F32, tag=f"win{s}")
            nc.sync.dma_start(out=wf, in_=wv[g])
            tpe = zpb[s].tile([128, GBLK, 128], F32, tag=f"zpb{s}")
            for b in range(GBLK):
                nc.tensor.transpose(tpe[:, b, :], wf[:, b, :], ident32[:, :])
            Gst32 = work.tile([128, GBLK, 128], F32, tag=f"gst32{s}")
            nc.vector.tensor_scalar(
                out=Gst32, in0=tpe, scalar1=1.0, scalar2=None, op0=OP.mult
            )
            yield

            Gst16 = None
            for i in range(L - 1, -1, -1):
                A16, aggr, rstd, xh16 = sv[i]
                # u = exp(-A) = 1 - sigmoid of the pre-softplus input;
                # independent of the matmul chain, issue early.
                u16 = work.tile([128, GBLK, 128], FP16, tag=f"u{s}")
                nc.scalar.activation(u16, A16, AF.Exp, scale=-1.0)

                gp = zpb[s].tile([128, GBLK, 128], F32, tag=f"zpb{s}")
                if i == L - 1:
                    for b in range(GBLK):
                        nc.tensor.matmul(
                            gp[:, b, :], Gst32[:, b, :], wb0[:, :],
                            start=True, stop=True,
                        )
                else:
                    for b in range(GBLK):
                        nc.tensor.matmul(
                            gp[:, b, :], Gst16[:, b, :], wbs[:, 3 - i, :],
                            start=True, stop=True,
                        )
                gd16 = work.tile([128, GBLK, 128], FP16, tag=f"gd{s}")
                nc.scalar.copy(gd16, gp)
                yield

                # srn = rstd*u - rstd (indep of gd) interleaved with the
                # m2 = sum(xhat*gd) reduction (indep of u) so the DVE wait
                # queue never fills with one not-ready dependency group.
                srn = work.tile([128, GBLK, 128], FP16, tag=f"srn{s}")
                pr = work.tile([128, GBLK, 128], FP16, tag=f"pr{s}")
                m2 = stats.tile([128, GBLK, 1], F32, tag=f"m2{s}")
                for b in range(2):
                    nc.vector.tensor_scalar(
                        out=srn[:, b, :], in0=u16[:, b, :],
                        scalar1=rstd[:, b, :], scalar2=rstd[:, b, :],
                        op0=OP.mult, op1=OP.subtract,
                    )
                    nc.vector.scalar_tensor_tensor(
                        out=pr[:, b, :], in0=xh16[:, b, :], scalar=1.0,
                        in1=gd16[:, b, :], op0=OP.mult, op1=OP.mult,
                        accum_out=m2[:, b, :],
                    )
                yield
                for b in range(2, GBLK):
                    nc.vector.tensor_scalar(
                        out=srn[:, b, :], in0=u16[:, b, :],
                        scalar1=rstd[:, b, :], scalar2=rstd[:, b, :],
                        op0=OP.mult, op1=OP.subtract,
                    )
                    nc.vector.scalar_tensor_tensor(
                        out=pr[:, b, :], in0=xh16[:, b, :], scalar=1.0,
                        in1=gd16[:, b, :], op0=OP.mult, op1=OP.mult,
                        accum_out=m2[:, b, :],
                    )
                q = stats.tile([128, GBLK, 1], F32, tag=f"q{s}")
                nc.gpsimd.tensor_scalar_mul(q, m2, 1.0 / H)
                yield

                # xm = xhat*q (4x tensor_scalar), dxn = xm - gd
                xm = work.tile([128, GBLK, 128], FP16, tag=f"xm{s}")
                for b in range(2):
                    nc.vector.tensor_scalar(
                        out=xm[:, b, :], in0=xh16[:, b, :],
                        scalar1=q[:, b, :], scalar2=None, op0=OP.mult,
                    )
                yield
                for b in range(2, GBLK):
                    nc.vector.tensor_scalar(
                        out=xm[:, b, :], in0=xh16[:, b, :],
                        scalar1=q[:, b, :], scalar2=None, op0=OP.mult,
                    )
                dxn = work.tile([128, GBLK, 128], FP16, tag=f"dxn{s}")
                nc.vector.tensor_tensor(
                    out=dxn, in0=xm, in1=gd16, op=OP.subtract
                )
                yield
                # dz = dxn * srn = (gd - xhat*m) * rstd * sigmoid
                dz = work.tile([128, GBLK, 128], FP16, tag=f"dz{s}")
                nc.gpsimd.tensor_tensor(out=dz, in0=dxn, in1=srn, op=OP.mult)
                yield
                tp = zpb[s].tile([128, GBLK, 128], FP16, tag=f"zpb{s}")
                for b in range(2):
                    nc.tensor.transpose(tp[:, b, :], dz[:, b, :], ident[:, :])
                yield
                for b in range(2, GBLK):
                    nc.tensor.transpose(tp[:, b, :], dz[:, b, :], ident[:, :])
                Gst16 = work.tile([128, GBLK, 128], FP16, tag=f"gst{s}")
                nc.vector.tensor_scalar(
                    out=Gst16, in0=tp, scalar1=1.0, scalar2=None, op0=OP.mult
                )
                yield

            gp = zpb[s].tile([128, GBLK, 128], F32, tag=f"zpb{s}")
            for b in range(GBLK):
                nc.tensor.matmul(
                    gp[:, b, :], Gst16[:, b, :], wbs[:, 4, :],
                    start=True, stop=True,
                )
            dwo = io.tile([128, GBLK, 128], F32, tag=f"dwout{s}")
            nc.scalar.copy(dwo, gp)
            nc.scalar.dma_start(out=dwv[g], in_=dwo)
            yield

        # Free-running phase-staggered pipeline: each stream s processes
        # groups s, s+NS, ... as one continuous fwd->bwd chain; streams are
        # primed with an emission-offset so their phases stay staggered and
        # every engine always sees ready work from some stream.
        assert NG % NS == 0

        def stream_gen(s):
            for g in range(s, NG, NS):
                sv = []
                yield from emit_fwd(g, s, sv)
                yield from emit_bwd(g, s, sv)

        gens = [iter(stream_gen(s)) for s in range(NS)]
        live = []
        PRIME = 12  # chunks of head-start between adjacent streams
        for s in range(NS):
            live.append(gens[s])
            for it in list(live):
                for _ in range(PRIME if it is gens[s] else 1):
                    try:
                        next(it)
                    except StopIteration:
                        if it in live:
                            live.remove(it)
                        break
        while live:
            for it in list(live):
                try:
                    next(it)
                except StopIteration:
                    live.remove(it)


def _host_precompute(t, W_in, b_in, fw, fb, gamma, beta, Wl, bl, W_out, b_out):
    t = np.asarray(t, dtype=np.float32).reshape(-1)[0]
    s = np.sin(t * np.asarray(fw, np.float32) + np.asarray(fb, np.float32))  # [L, H]
    Wl = np.asarray(Wl, np.float32)
    gamma = np.asarray(gamma, np.float32)
    beta = np.asarray(beta, np.float32)
    bl = np.asarray(bl, np.float32)
    W_in = np.asarray(W_in, np.float32)
    W_out = np.asarray(W_out, np.float32)
    b_in = np.asarray(b_in, np.float32)
    b_out = np.asarray(b_out, np.float32)

    Wg = [Wl[i] * gamma[i][None, :] for i in range(L)]          # [H, H]
    bg = [bl[i] + Wl[i] @ beta[i] for i in range(L)]            # [H]

    # fuse h4->y: y = xhat3 @ (W_out@Wg3).T + (b_out + W_out@bg3)
    M2 = (W_out.astype(np.float64) @ Wg[L - 1].astype(np.float64)).astype(np.float32)
    c = np.zeros((5, 128), np.float32)
    c[0] = b_in + s[0]
    for i in range(1, L):
        c[i] = bg[i - 1] + s[i]
    c[4] = b_out + W_out @ bg[L - 1]
    WF = np.stack(
        [W_in.T] + [Wg[i].T for i in range(L - 1)] + [M2.T], axis=0
    )  # [5, K, N]
    Wc = [Wg[i] - Wg[i].mean(axis=1, keepdims=True) for i in range(L - 1)]
    M2n = -M2
    M2c = M2n - M2n.mean(axis=1, keepdims=True)
    WB = np.stack([M2c, Wc[2], Wc[1], Wc[0], W_in], axis=0)

    WF16 = np.ascontiguousarray(np.transpose(WF, (1, 0, 2))).astype(np.float16)
    WF0 = np.ascontiguousarray(W_in.T)  # [K, N] f32
    WB16 = np.ascontiguousarray(np.transpose(WB, (1, 0, 2))).astype(np.float16)
    WB0 = np.ascontiguousarray(M2c)  # f32
    GBLK = 4
    CB = np.tile(c, (1, GBLK)).astype(np.float16)[None, :, :]  # [1, 5, GBLK*128]
    CB0 = np.tile(c[0:1, :], (1, GBLK)).astype(np.float32)     # [1, GBLK*128]
    ONES = np.ones((1, 128), np.float16)
    ONES32 = np.ones((1, 128), np.float32)
    EYE = np.eye(128, dtype=np.float16)
    EYE32 = np.eye(128, dtype=np.float32)
    return WF16, WF0, WB16, WB0, CB, CB0, ONES, ONES32, EYE, EYE32


_NC_CACHE = {}


def _get_nc(R, GBLK):
    key = (R, GBLK)
    if key not in _NC_CACHE:
        nc = bacc.Bacc("TRN2")
        _emit(nc, R, GBLK)
        nc.finalize()
        _NC_CACHE[key] = nc
    return _NC_CACHE[key]


def _run(p, w, consts, R, GBLK, n_cores):
    WF16, WF0, WB16, WB0, CB, CB0, ONES, ONES32, EYE, EYE32 = consts
    nc = _get_nc(R, GBLK)
    in_maps = []
    for k in range(n_cores):
        in_maps.append(
            {
                "p": np.ascontiguousarray(p[k * R : (k + 1) * R]),
                "w": np.ascontiguousarray(w[k * R : (k + 1) * R]),
                "wf": WF16,
                "wf0": WF0,
                "wb": WB16,
                "wb0": WB0,
                "cb": CB,
                "cb0": CB0,
                "ones1": ONES,
                "ones1_32": ONES32,
                "ident": EYE,
                "ident32": EYE32,
            }
        )
    res = run_bass_kernel_spmd(nc, in_maps, core_ids=list(range(n_cores)))
    dp = np.concatenate([r["dp"] for r in res.results], axis=0)
    dw = np.concatenate([r["dw"] for r in res.results], axis=0)
    return dp, dw


def kernel(t, p, w, W_in, b_in, fw, fb, gamma, beta, Wl, bl, W_out, b_out):
    consts = _host_precompute(
        t, W_in, b_in, fw, fb, gamma, beta, Wl, bl, W_out, b_out
    )
    p = np.asarray(p, np.float32)
    w = np.asarray(w, np.float32)
    R = p.shape[0] // NCORES
    dp, dw = _run(p, w, consts, R, GBLK=4, n_cores=NCORES)
    return dp, dw


# revision 51
# speedup vs baseline: 2.0022x; 1.0308x over previous
"""Trainium2 Bass kernel for nn_CLNF_54769422959177.

Computes (dp, dw) where dp = vf(p) (4-layer VectorField MLP forward) and
dw = -vjp(vf, p)(w), data-parallel over 8 NeuronCores.

v3 design (1178909 -> 606915 ns vs the v1 baseline):
- A single manual InstLoadActFuncSet(natural_log_exp_and_others) at program
  start: every ACT func used (Exp/Ln/Copy) lives in that one table, so the
  finalize pass inserts no further table loads (v1 thrashed 443 loads
  = 568us on the ACT engine).
- LN stats via per-block bn_stats/bn_aggr (mean+var in one DVE pass)
  instead of Square + two reduces; rstd = exp(-0.5*ln(var+eps)).
- Backward in xhat-form: dz = (gd - xhat*m)*rstd*sigmoid, with m2 from
  scalar_tensor_tensor+accum and sigmoid = 1 - exp(-A) from the saved
  softplus output A (no recompute of the forward pre-activation).
- fp16 tensor_scalar ops (4x DVE mode) for xhat/srn/xm with per-block
  [128,1] stat scalars; engine split: ACT = exp/ln + PSUM evacuations,
  DVE = bn/stt/ts/tt + transpose copies, Pool = dz + small stat ops.
- f32 entry: p and w are PE-transposed and matmul'ed in f32 directly (PE
  has slack), skipping the f32->fp16 cast pass of v1.
- NS=4 phase-staggered free-running streams; one PSUM bank per stream per
  direction (entry transposes, matmuls and activation transposes share a
  single-buffer ring - all chain-serial within a stream); batched bias
  matmul (one wide K=1 matmul, no ones<->Xst ldweights ping-pong); per-block
  instruction quads split/interleaved across yields so the 4-deep in-order
  engine wait queues don't head-of-line block on one late dependency;
  output-store DMAs issued from the producing engine (ACT) so the SP
  sequencer never parks on result availability.
"""

import numpy as np

import concourse.bass as bass
from concourse import bacc
import concourse.tile as tile
from concourse import mybir
from concourse.bass_utils import run_bass_kernel_spmd

B, D, H, L = 131072, 128, 128, 4
NCORES = 8
LN_EPS = 1e-5
FP16 = mybir.dt.float16
F32 = mybir.dt.float32
AF = mybir.ActivationFunctionType
OP = mybir.AluOpType
ACT_TABLE_NL_EXP = 6  # natural_log_exp_and_others in cayman act_info.json

TileCtx = tile.TileContext


def _emit(nc, R, GBLK):
    """Emit the per-core program: R rows, blocks of 128 rows, GBLK blocks/group."""
    NG = R // (GBLK * 128)
    assert NG * GBLK * 128 == R

    p_in = nc.dram_tensor("p", [R, D], F32, kind="ExternalInput")
    w_in = nc.dram_tensor("w", [R, D], F32, kind="ExternalInput")
    # moving weights fwd: [K, 5, N] = {W_in.T, Wg0.T, Wg1.T, Wg2.T, M2.T}
    wf_in = nc.dram_tensor("wf", [128, 5, 128], FP16, kind="ExternalInput")
    wf0_in = nc.dram_tensor("wf0", [128, 128], F32, kind="ExternalInput")
    # moving weights bwd: {M2c, Wc2, Wc1, Wc0, W_in}
    wb_in = nc.dram_tensor("wb", [128, 5, 128], FP16, kind="ExternalInput")
    wb0_in = nc.dram_tensor("wb0", [128, 128], F32, kind="ExternalInput")
    cb_in = nc.dram_tensor("cb", [1, 5, GBLK * 128], FP16, kind="ExternalInput")
    cb0_in = nc.dram_tensor("cb0", [1, GBLK * 128], F32, kind="ExternalInput")
    ones_in = nc.dram_tensor("ones1", [1, 128], FP16, kind="ExternalInput")
    ones32_in = nc.dram_tensor("ones1_32", [1, 128], F32, kind="ExternalInput")
    id_in = nc.dram_tensor("ident", [128, 128], FP16, kind="ExternalInput")
    id32_in = nc.dram_tensor("ident32", [128, 128], F32, kind="ExternalInput")
    dp_out = nc.dram_tensor("dp", [R, D], F32, kind="ExternalOutput")
    dw_out = nc.dram_tensor("dw", [R, D], F32, kind="ExternalOutput")

    pv = p_in[:, :].rearrange("(g b p) d -> g p b d", p=128, b=GBLK)
    wv = w_in[:, :].rearrange("(g b p) d -> g p b d", p=128, b=GBLK)
    dpv = dp_out[:, :].rearrange("(g b p) d -> g p b d", p=128, b=GBLK)
    dwv = dw_out[:, :].rearrange("(g b p) d -> g p b d", p=128, b=GBLK)

    from contextlib import ExitStack

    with TileCtx(nc) as tc, ExitStack() as ctx:
        NS = 4
        consts = ctx.enter_context(tc.tile_pool(name="consts", bufs=1))
        io = ctx.enter_context(tc.tile_pool(name="io", bufs=2))
        work = ctx.enter_context(tc.tile_pool(name="work", bufs=1))
        saves = ctx.enter_context(tc.tile_pool(name="saves", bufs=2))
        stats = ctx.enter_context(tc.tile_pool(name="stats", bufs=2))
        # One PSUM bank per stream per direction; entry transposes, layer
        # matmuls and activation transposes all rotate through the same
        # single-buffer ring (their uses are chain-serial within a stream).
        zpf = [
            ctx.enter_context(tc.tile_pool(name=f"zpf{s}", bufs=1, space="PSUM"))
            for s in range(NS)
        ]
        zpb = [
            ctx.enter_context(tc.tile_pool(name=f"zpb{s}", bufs=1, space="PSUM"))
            for s in range(NS)
        ]

        wfs = consts.tile([128, 5, 128], FP16, tag="wfs")
        wf0 = consts.tile([128, 128], F32, tag="wf0")
        wbs = consts.tile([128, 5, 128], FP16, tag="wbs")
        wb0 = consts.tile([128, 128], F32, tag="wb0")
        cbs = consts.tile([1, 5, GBLK * 128], FP16, tag="cbs")
        cb0 = consts.tile([1, GBLK * 128], F32, tag="cb0")
        ones1 = consts.tile([1, 128], FP16, tag="ones1")
        ones1_32 = consts.tile([1, 128], F32, tag="ones1_32")
        ident = consts.tile([128, 128], FP16, tag="ident")
        ident32 = consts.tile([128, 128], F32, tag="ident32")
        epsb = consts.tile([128, 1], F32, tag="epsb")
        nc.vector.memset(epsb, LN_EPS)
        nc.gpsimd.dma_start(out=wfs[:], in_=wf_in[:, :, :])
        nc.gpsimd.dma_start(out=wf0[:], in_=wf0_in[:, :])
        nc.gpsimd.dma_start(out=wbs[:], in_=wb_in[:, :, :])
        nc.gpsimd.dma_start(out=wb0[:], in_=wb0_in[:, :])
        nc.gpsimd.dma_start(out=cbs[:], in_=cb_in[:, :, :])
        nc.gpsimd.dma_start(out=cb0[:], in_=cb0_in[:, :])
        nc.gpsimd.dma_start(out=ones1[:], in_=ones_in[:, :])
        nc.gpsimd.dma_start(out=ones1_32[:], in_=ones32_in[:, :])
        nc.gpsimd.dma_start(out=ident[:], in_=id_in[:, :])
        nc.gpsimd.dma_start(out=ident32[:], in_=id32_in[:, :])

        # One activation table covering Exp, Ln, Copy: loaded once, the
        # finalize fixpoint then inserts no per-activation loads.
        ld = mybir.InstLoadActFuncSet(
            name=nc.get_next_instruction_name(), ins=[], outs=[]
        )
        ld.act_func_set_id = ACT_TABLE_NL_EXP
        nc.scalar.add_instruction(ld)

        def emit_fwd(g, s, out):
            """Generator: forward for group g on stream s; appends saves."""
            pf = io.tile([128, GBLK, 128], F32, tag=f"pin{s}")
            nc.sync.dma_start(out=pf, in_=pv[g])
            # entry: f32 transpose + f32 copy to SBUF (no fp16 cast pass)
            tpe = zpf[s].tile([128, GBLK, 128], F32, tag=f"zpf{s}")
            for b in range(GBLK):
                nc.tensor.transpose(tpe[:, b, :], pf[:, b, :], ident32[:, :])
            Xst32 = work.tile([128, GBLK, 128], F32, tag=f"xst32{s}")
            nc.scalar.copy(Xst32, tpe)
            yield

            Xst16 = None
            for i in range(L):
                zp = zpf[s].tile([128, GBLK, 128], F32, tag=f"zpf{s}")
                if i == 0:
                    nc.tensor.matmul(
                        zp[:, :, :], ones1_32[:, :], cb0[:, :],
                        start=True, stop=False, skip_group_check=True,
                    )
                    for b in range(GBLK):
                        nc.tensor.matmul(
                            zp[:, b, :], Xst32[:, b, :], wf0[:, :],
                            start=False, stop=True, skip_group_check=True,
                        )
                else:
                    nc.tensor.matmul(
                        zp[:, :, :], ones1[:, :], cbs[:, i, :],
                        start=True, stop=False, skip_group_check=True,
                    )
                    for b in range(GBLK):
                        nc.tensor.matmul(
                            zp[:, b, :], Xst16[:, b, :], wfs[:, i, :],
                            start=False, stop=True, skip_group_check=True,
                        )

                E = work.tile([128, GBLK, 128], F32, tag=f"E{s}")
                nc.scalar.activation(E, zp, AF.Exp)
                A16 = saves.tile([128, GBLK, 128], FP16, tag=f"A{i}{s}")
                nc.scalar.activation(A16, E, AF.Ln, bias=1.0)
                yield

                st6 = stats.tile([128, GBLK, 6], F32, tag=f"st6{s}")
                aggr = saves.tile([128, GBLK, 2], F32, tag=f"ag{i}{s}")
                for b in range(2):
                    nc.vector.bn_stats(out=st6[:, b, :], in_=A16[:, b, :])
                yield
                for b in range(2, GBLK):
                    nc.vector.bn_stats(out=st6[:, b, :], in_=A16[:, b, :])
                for b in range(2):
                    nc.vector.bn_aggr(out=aggr[:, b, :], in_=st6[:, b, :])
                yield
                for b in range(2, GBLK):
                    nc.vector.bn_aggr(out=aggr[:, b, :], in_=st6[:, b, :])
                lnv = stats.tile([128, GBLK, 1], F32, tag=f"lnv{s}")
                nc.scalar.activation(
                    lnv, aggr[:, :, 1:2], AF.Ln, bias=epsb[:, :]
                )
                rstd = saves.tile([128, GBLK, 1], F32, tag=f"rs{i}{s}")
                nc.scalar.activation(rstd, lnv, AF.Exp, scale=-0.5)
                yield

                xh16 = saves.tile([128, GBLK, 128], FP16, tag=f"xh{i}{s}")
                for b in range(2):
                    nc.vector.tensor_scalar(
                        out=xh16[:, b, :], in0=A16[:, b, :],
                        scalar1=aggr[:, b, 0:1], scalar2=rstd[:, b, :],
                        op0=OP.subtract, op1=OP.mult,
                    )
                yield
                tp = zpf[s].tile([128, GBLK, 128], FP16, tag=f"zpf{s}")
                for b in range(2, GBLK):
                    nc.vector.tensor_scalar(
                        out=xh16[:, b, :], in0=A16[:, b, :],
                        scalar1=aggr[:, b, 0:1], scalar2=rstd[:, b, :],
                        op0=OP.subtract, op1=OP.mult,
                    )
                for b in range(2):
                    nc.tensor.transpose(tp[:, b, :], xh16[:, b, :], ident[:, :])
                yield
                for b in range(2, GBLK):
                    nc.tensor.transpose(tp[:, b, :], xh16[:, b, :], ident[:, :])
                Xst16 = work.tile([128, GBLK, 128], FP16, tag=f"xst{s}")
                nc.vector.tensor_scalar(
                    out=Xst16, in0=tp, scalar1=1.0, scalar2=None, op0=OP.mult
                )
                out.append((A16, aggr, rstd, xh16))
                yield

            zp = zpf[s].tile([128, GBLK, 128], F32, tag=f"zpf{s}")
            nc.tensor.matmul(
                zp[:, :, :], ones1[:, :], cbs[:, 4, :],
                start=True, stop=False, skip_group_check=True,
            )
            for b in range(GBLK):
                nc.tensor.matmul(
                    zp[:, b, :], Xst16[:, b, :], wfs[:, 4, :],
                    start=False, stop=True, skip_group_check=True,
                )
            yo = io.tile([128, GBLK, 128], F32, tag=f"yout{s}", bufs=1)
            nc.scalar.copy(yo, zp)
            # issue the store from ACT (the producer) so the DMA wait is
            # satisfied by construction and never parks the SP sequencer
            nc.scalar.dma_start(out=dpv[g], in_=yo)
            yield

        def emit_bwd(g, s, sv):
            wf = io.tile([128, GBLK, 128], F32, tag=f"win{s}")
            nc.sync.dma_start(out=wf, in_=wv[g])
            tpe = zpb[s].tile([128, GBLK, 128], F32, tag=f"zpb{s}")
            for b in range(GBLK):
                nc.tensor.transpose(tpe[:, b, :], wf[:, b, :], ident32[:, :])
            Gst32 = work.tile([128, GBLK, 128], F32, tag=f"gst32{s}")
            nc.vector.tensor_scalar(
                out=Gst32, in0=tpe, scalar1=1.0, scalar2=None, op0=OP.mult
            )
            yield

            Gst16 = None
            for i in range(L - 1, -1, -1):
                A16, aggr, rstd, xh16 = sv[i]
                # u = exp(-A) = 1 - sigmoid of the pre-softplus input;
                # independent of the matmul chain, issue early.
                u16 = work.tile([128, GBLK, 128], FP16, tag=f"u{s}", bufs=2)
                nc.scalar.activation(u16, A16, AF.Exp, scale=-1.0)

                gp = zpb[s].tile([128, GBLK, 128], F32, tag=f"zpb{s}")
                if i == L - 1:
                    for b in range(GBLK):
                        nc.tensor.matmul(
                            gp[:, b, :], Gst32[:, b, :], wb0[:, :],
                            start=True, stop=True,
                        )
                else:
                    for b in range(GBLK):
                        nc.tensor.matmul(
                            gp[:, b, :], Gst16[:, b, :], wbs[:, 3 - i, :],
                            start=True, stop=True,
                        )
                gd16 = work.tile([128, GBLK, 128], FP16, tag=f"gd{s}", bufs=2)
                nc.scalar.copy(gd16, gp)
                yield

                # srn = rstd*u - rstd (indep of gd) interleaved with the
                # m2 = sum(xhat*gd) reduction (indep of u) so the DVE wait
                # queue never fills with one not-ready dependency group.
                srn = work.tile([128, GBLK, 128], FP16, tag=f"srn{s}", bufs=2)
                pr = work.tile([128, GBLK, 128], FP16, tag=f"pr{s}")
                m2 = stats.tile([128, GBLK, 1], F32, tag=f"m2{s}")
                for b in range(2):
                    nc.gpsimd.tensor_scalar(
                        out=srn[:, b, :], in0=u16[:, b, :],
                        scalar1=rstd[:, b, :], scalar2=rstd[:, b, :],
                        op0=OP.mult, op1=OP.subtract,
                    )
                    nc.vector.scalar_tensor_tensor(
                        out=pr[:, b, :], in0=xh16[:, b, :], scalar=1.0 / H,
                        in1=gd16[:, b, :], op0=OP.mult, op1=OP.mult,
                        accum_out=m2[:, b, :],
                    )
                yield
                for b in range(2, GBLK):
                    nc.gpsimd.tensor_scalar(
                        out=srn[:, b, :], in0=u16[:, b, :],
                        scalar1=rstd[:, b, :], scalar2=rstd[:, b, :],
                        op0=OP.mult, op1=OP.subtract,
                    )
                    nc.vector.scalar_tensor_tensor(
                        out=pr[:, b, :], in0=xh16[:, b, :], scalar=1.0 / H,
                        in1=gd16[:, b, :], op0=OP.mult, op1=OP.mult,
                        accum_out=m2[:, b, :],
                    )
                yield

                # xm = xhat*q (4x tensor_scalar), dxn = xm - gd
                xm = work.tile([128, GBLK, 128], FP16, tag=f"xm{s}")
                for b in range(2):
                    nc.vector.tensor_scalar(
                        out=xm[:, b, :], in0=xh16[:, b, :],
                        scalar1=m2[:, b, :], scalar2=None, op0=OP.mult,
                    )
                yield
                for b in range(2, GBLK):
                    nc.vector.tensor_scalar(
                        out=xm[:, b, :], in0=xh16[:, b, :],
                        scalar1=m2[:, b, :], scalar2=None, op0=OP.mult,
                    )
                dxn = work.tile([128, GBLK, 128], FP16, tag=f"dxn{s}", bufs=2)
                nc.vector.tensor_tensor(
                    out=dxn, in0=xm, in1=gd16, op=OP.subtract
                )
                yield
                # dz = dxn * srn = (gd - xhat*m) * rstd * sigmoid
                dz = work.tile([128, GBLK, 128], FP16, tag=f"dz{s}", bufs=2)
                nc.vector.tensor_tensor(out=dz, in0=dxn, in1=srn, op=OP.mult)
                yield
                tp = zpb[s].tile([128, GBLK, 128], FP16, tag=f"zpb{s}")
                for b in range(2):
                    nc.tensor.transpose(tp[:, b, :], dz[:, b, :], ident[:, :])
                yield
                for b in range(2, GBLK):
                    nc.tensor.transpose(tp[:, b, :], dz[:, b, :], ident[:, :])
                Gst16 = work.tile([128, GBLK, 128], FP16, tag=f"gst{s}")
                nc.vector.tensor_scalar(
                    out=Gst16, in0=tp, scalar1=1.0, scalar2=None, op0=OP.mult
                )
                yield

            gp = zpb[s].tile([128, GBLK, 128], F32, tag=f"zpb{s}")
            for b in range(GBLK):
                nc.tensor.matmul(
                    gp[:, b, :], Gst16[:, b, :], wbs[:, 4, :],
                    start=True, stop=True,
                )
            dwo = io.tile([128, GBLK, 128], F32, tag=f"dwout{s}", bufs=1)
            nc.scalar.copy(dwo, gp)
            nc.scalar.dma_start(out=dwv[g], in_=dwo)
            yield

        # Free-running phase-staggered pipeline: each stream s processes
        # groups s, s+NS, ... as one continuous fwd->bwd chain; streams are
        # primed with an emission-offset so their phases stay staggered and
        # every engine always sees ready work from some stream.
        assert NG % NS == 0

        def stream_gen(s):
            for g in range(s, NG, NS):
                sv = []
                yield from emit_fwd(g, s, sv)
                yield from emit_bwd(g, s, sv)

        gens = [iter(stream_gen(s)) for s in range(NS)]
        live = []
        PRIME = 8  # chunks of head-start between adjacent streams
        for s in range(NS):
            live.append(gens[s])
            for it in list(live):
                for _ in range(PRIME if it is gens[s] else 1):
                    try:
                        next(it)
                    except StopIteration:
                        if it in live:
                            live.remove(it)
                        break
        while live:
            for it in list(live):
                try:
                    next(it)
                except StopIteration:
                    live.remove(it)


def _host_precompute(t, W_in, b_in, fw, fb, gamma, beta, Wl, bl, W_out, b_out):
    t = np.asarray(t, dtype=np.float32).reshape(-1)[0]
    s = np.sin(t * np.asarray(fw, np.float32) + np.asarray(fb, np.float32))  # [L, H]
    Wl = np.asarray(Wl, np.float32)
    gamma = np.asarray(gamma, np.float32)
    beta = np.asarray(beta, np.float32)
    bl = np.asarray(bl, np.float32)
    W_in = np.asarray(W_in, np.float32)
    W_out = np.asarray(W_out, np.float32)
    b_in = np.asarray(b_in, np.float32)
    b_out = np.asarray(b_out, np.float32)

    Wg = [Wl[i] * gamma[i][None, :] for i in range(L)]          # [H, H]
    bg = [bl[i] + Wl[i] @ beta[i] for i in range(L)]            # [H]

    # fuse h4->y: y = xhat3 @ (W_out@Wg3).T + (b_out + W_out@bg3)
    M2 = (W_out.astype(np.float64) @ Wg[L - 1].astype(np.float64)).astype(np.float32)
    c = np.zeros((5, 128), np.float32)
    c[0] = b_in + s[0]
    for i in range(1, L):
        c[i] = bg[i - 1] + s[i]
    c[4] = b_out + W_out @ bg[L - 1]
    WF = np.stack(
        [W_in.T] + [Wg[i].T for i in range(L - 1)] + [M2.T], axis=0
    )  # [5, K, N]
    Wc = [Wg[i] - Wg[i].mean(axis=1, keepdims=True) for i in range(L - 1)]
    M2n = -M2
    M2c = M2n - M2n.mean(axis=1, keepdims=True)
    WB = np.stack([M2c, Wc[2], Wc[1], Wc[0], W_in], axis=0)

    WF16 = np.ascontiguousarray(np.transpose(WF, (1, 0, 2))).astype(np.float16)
    WF0 = np.ascontiguousarray(W_in.T)  # [K, N] f32
    WB16 = np.ascontiguousarray(np.transpose(WB, (1, 0, 2))).astype(np.float16)
    WB0 = np.ascontiguousarray(M2c)  # f32
    GBLK = 4
    CB = np.tile(c, (1, GBLK)).astype(np.float16)[None, :, :]  # [1, 5, GBLK*128]
    CB0 = np.tile(c[0:1, :], (1, GBLK)).astype(np.float32)     # [1, GBLK*128]
    ONES = np.ones((1, 128), np.float16)
    ONES32 = np.ones((1, 128), np.float32)
    EYE = np.eye(128, dtype=np.float16)
    EYE32 = np.eye(128, dtype=np.float32)
    return WF16, WF0, WB16, WB0, CB, CB0, ONES, ONES32, EYE, EYE32


_NC_CACHE = {}


def _get_nc(R, GBLK):
    key = (R, GBLK)
    if key not in _NC_CACHE:
        nc = bacc.Bacc("TRN2")
        _emit(nc, R, GBLK)
        nc.finalize()
        _NC_CACHE[key] = nc
    return _NC_CACHE[key]


def _run(p, w, consts, R, GBLK, n_cores):
    WF16, WF0, WB16, WB0, CB, CB0, ONES, ONES32, EYE, EYE32 = consts
    nc = _get_nc(R, GBLK)
    in_maps = []
    for k in range(n_cores):
        in_maps.append(
            {
                "p": np.ascontiguousarray(p[k * R : (k + 1) * R]),
                "w": np.ascontiguousarray(w[k * R : (k + 1) * R]),
                "wf": WF16,
                "wf0": WF0,
                "wb": WB16,
                "wb0": WB0,
                "cb": CB,
                "cb0": CB0,
                "ones1": ONES,
                "ones1_32": ONES32,
                "ident": EYE,
                "ident32": EYE32,
            }
        )
    res = run_bass_kernel_spmd(nc, in_maps, core_ids=list(range(n_cores)))
    dp = np.concatenate([r["dp"] for r in res.results], axis=0)
    dw = np.concatenate([r["dw"] for r in res.results], axis=0)
    return dp, dw


def kernel(t, p, w, W_in, b_in, fw, fb, gamma, beta, Wl, bl, W_out, b_out):
    consts = _host_precompute(
        t, W_in, b_in, fw, fb, gamma, beta, Wl, bl, W_out, b_out
    )
    p = np.asarray(p, np.float32)
    w = np.asarray(w, np.float32)
    R = p.shape[0] // NCORES
    dp, dw = _run(p, w, consts, R, GBLK=4, n_cores=NCORES)
    return dp, dw


# revision 59
# speedup vs baseline: 2.0220x; 1.0099x over previous
"""Trainium2 Bass kernel for nn_CLNF_54769422959177.

Computes (dp, dw) where dp = vf(p) (4-layer VectorField MLP forward) and
dw = -vjp(vf, p)(w), data-parallel over 8 NeuronCores.

v3 design (1178909 -> 606915 ns vs the v1 baseline):
- A single manual InstLoadActFuncSet(natural_log_exp_and_others) at program
  start: every ACT func used (Exp/Ln/Copy) lives in that one table, so the
  finalize pass inserts no further table loads (v1 thrashed 443 loads
  = 568us on the ACT engine).
- LN stats via per-block bn_stats/bn_aggr (mean+var in one DVE pass)
  instead of Square + two reduces; rstd = exp(-0.5*ln(var+eps)).
- Backward in xhat-form: dz = (gd - xhat*m)*rstd*sigmoid, with m2 from
  scalar_tensor_tensor+accum and sigmoid = 1 - exp(-A) from the saved
  softplus output A (no recompute of the forward pre-activation).
- fp16 tensor_scalar ops (4x DVE mode) for xhat/srn/xm with per-block
  [128,1] stat scalars; engine split: ACT = exp/ln + PSUM evacuations,
  DVE = bn/stt/ts/tt + transpose copies, Pool = dz + small stat ops.
- f32 entry: p and w are PE-transposed and matmul'ed in f32 directly (PE
  has slack), skipping the f32->fp16 cast pass of v1.
- NS=4 phase-staggered free-running streams; one PSUM bank per stream per
  direction (entry transposes, matmuls and activation transposes share a
  single-buffer ring - all chain-serial within a stream); batched bias
  matmul (one wide K=1 matmul, no ones<->Xst ldweights ping-pong); per-block
  instruction quads split/interleaved across yields so the 4-deep in-order
  engine wait queues don't head-of-line block on one late dependency;
  output-store DMAs issued from the producing engine (ACT) so the SP
  sequencer never parks on result availability.
"""

import numpy as np

import concourse.bass as bass
from concourse import bacc
import concourse.tile as tile
from concourse import mybir
from concourse.bass_utils import run_bass_kernel_spmd

B, D, H, L = 131072, 128, 128, 4
NCORES = 8
LN_EPS = 1e-5
FP16 = mybir.dt.float16
F32 = mybir.dt.float32
AF = mybir.ActivationFunctionType
OP = mybir.AluOpType
ACT_TABLE_NL_EXP = 6  # natural_log_exp_and_others in cayman act_info.json

TileCtx = tile.TileContext


def _emit(nc, R, GBLK):
    """Emit the per-core program: R rows, blocks of 128 rows, GBLK blocks/group."""
    NG = R // (GBLK * 128)
    assert NG * GBLK * 128 == R

    p_in = nc.dram_tensor("p", [R, D], F32, kind="ExternalInput")
    w_in = nc.dram_tensor("w", [R, D], F32, kind="ExternalInput")
    # moving weights fwd: [K, 5, N] = {W_in.T, Wg0.T, Wg1.T, Wg2.T, M2.T}
    wf_in = nc.dram_tensor("wf", [128, 5, 128], FP16, kind="ExternalInput")
    wf0_in = nc.dram_tensor("wf0", [128, 128], F32, kind="ExternalInput")
    # moving weights bwd: {M2c, Wc2, Wc1, Wc0, W_in}
    wb_in = nc.dram_tensor("wb", [128, 5, 128], FP16, kind="ExternalInput")
    wb0_in = nc.dram_tensor("wb0", [128, 128], F32, kind="ExternalInput")
    cb_in = nc.dram_tensor("cb", [1, 5, GBLK * 128], FP16, kind="ExternalInput")
    cb0_in = nc.dram_tensor("cb0", [1, GBLK * 128], F32, kind="ExternalInput")
    ones_in = nc.dram_tensor("ones1", [1, 128], FP16, kind="ExternalInput")
    ones32_in = nc.dram_tensor("ones1_32", [1, 128], F32, kind="ExternalInput")
    id_in = nc.dram_tensor("ident", [128, 128], FP16, kind="ExternalInput")
    id32_in = nc.dram_tensor("ident32", [128, 128], F32, kind="ExternalInput")
    dp_out = nc.dram_tensor("dp", [R, D], F32, kind="ExternalOutput")
    dw_out = nc.dram_tensor("dw", [R, D], F32, kind="ExternalOutput")

    pv = p_in[:, :].rearrange("(g b p) d -> g p b d", p=128, b=GBLK)
    wv = w_in[:, :].rearrange("(g b p) d -> g p b d", p=128, b=GBLK)
    dpv = dp_out[:, :].rearrange("(g b p) d -> g p b d", p=128, b=GBLK)
    dwv = dw_out[:, :].rearrange("(g b p) d -> g p b d", p=128, b=GBLK)

    from contextlib import ExitStack

    with TileCtx(nc) as tc, ExitStack() as ctx:
        NS = 4
        consts = ctx.enter_context(tc.tile_pool(name="consts", bufs=1))
        io = ctx.enter_context(tc.tile_pool(name="io", bufs=2))
        work = ctx.enter_context(tc.tile_pool(name="work", bufs=1))
        saves = ctx.enter_context(tc.tile_pool(name="saves", bufs=2))
        stats = ctx.enter_context(tc.tile_pool(name="stats", bufs=2))
        # One PSUM bank per stream per direction; entry transposes, layer
        # matmuls and activation transposes all rotate through the same
        # single-buffer ring (their uses are chain-serial within a stream).
        zpf = [
            ctx.enter_context(tc.tile_pool(name=f"zpf{s}", bufs=1, space="PSUM"))
            for s in range(NS)
        ]
        zpb = [
            ctx.enter_context(tc.tile_pool(name=f"zpb{s}", bufs=1, space="PSUM"))
            for s in range(NS)
        ]

        wfs = consts.tile([128, 5, 128], FP16, tag="wfs")
        wf0 = consts.tile([128, 128], F32, tag="wf0")
        wbs = consts.tile([128, 5, 128], FP16, tag="wbs")
        wb0 = consts.tile([128, 128], F32, tag="wb0")
        cbs = consts.tile([1, 5, GBLK * 128], FP16, tag="cbs")
        cb0 = consts.tile([1, GBLK * 128], F32, tag="cb0")
        ones1 = consts.tile([1, 128], FP16, tag="ones1")
        ones1_32 = consts.tile([1, 128], F32, tag="ones1_32")
        ident = consts.tile([128, 128], FP16, tag="ident")
        ident32 = consts.tile([128, 128], F32, tag="ident32")
        epsb = consts.tile([128, 1], F32, tag="epsb")
        nc.vector.memset(epsb, LN_EPS)
        nc.gpsimd.dma_start(out=wfs[:], in_=wf_in[:, :, :])
        nc.gpsimd.dma_start(out=wf0[:], in_=wf0_in[:, :])
        nc.gpsimd.dma_start(out=wbs[:], in_=wb_in[:, :, :])
        nc.gpsimd.dma_start(out=wb0[:], in_=wb0_in[:, :])
        nc.gpsimd.dma_start(out=cbs[:], in_=cb_in[:, :, :])
        nc.gpsimd.dma_start(out=cb0[:], in_=cb0_in[:, :])
        nc.gpsimd.dma_start(out=ones1[:], in_=ones_in[:, :])
        nc.gpsimd.dma_start(out=ones1_32[:], in_=ones32_in[:, :])
        nc.gpsimd.dma_start(out=ident[:], in_=id_in[:, :])
        nc.gpsimd.dma_start(out=ident32[:], in_=id32_in[:, :])

        # One activation table covering Exp, Ln, Copy: loaded once, the
        # finalize fixpoint then inserts no per-activation loads.
        ld = mybir.InstLoadActFuncSet(
            name=nc.get_next_instruction_name(), ins=[], outs=[]
        )
        ld.act_func_set_id = ACT_TABLE_NL_EXP
        nc.scalar.add_instruction(ld)

        def emit_fwd(g, s, out):
            """Generator: forward for group g on stream s; appends saves."""
            pf = io.tile([128, GBLK, 128], F32, tag=f"pin{s}")
            nc.sync.dma_start(out=pf, in_=pv[g])
            # entry: f32 transpose + f32 copy to SBUF (no fp16 cast pass)
            tpe = zpf[s].tile([128, GBLK, 128], F32, tag=f"zpf{s}")
            for b in range(GBLK):
                nc.tensor.transpose(tpe[:, b, :], pf[:, b, :], ident32[:, :])
            Xst32 = work.tile([128, GBLK, 128], F32, tag=f"xst32{s}")
            nc.scalar.copy(Xst32, tpe)
            yield

            Xst16 = None
            for i in range(L):
                zp = zpf[s].tile([128, GBLK, 128], F32, tag=f"zpf{s}")
                if i == 0:
                    nc.tensor.matmul(
                        zp[:, :, :], ones1_32[:, :], cb0[:, :],
                        start=True, stop=False, skip_group_check=True,
                    )
                    for b in range(GBLK):
                        nc.tensor.matmul(
                            zp[:, b, :], Xst32[:, b, :], wf0[:, :],
                            start=False, stop=True, skip_group_check=True,
                        )
                else:
                    nc.tensor.matmul(
                        zp[:, :, :], ones1[:, :], cbs[:, i, :],
                        start=True, stop=False, skip_group_check=True,
                    )
                    for b in range(GBLK):
                        nc.tensor.matmul(
                            zp[:, b, :], Xst16[:, b, :], wfs[:, i, :],
                            start=False, stop=True, skip_group_check=True,
                        )

                E = work.tile([128, GBLK, 128], F32, tag=f"E{s}")
                nc.scalar.activation(E, zp, AF.Exp)
                A16 = saves.tile([128, GBLK, 128], FP16, tag=f"A{i}{s}")
                nc.scalar.activation(A16, E, AF.Ln, bias=1.0)
                yield

                st6 = stats.tile([128, GBLK, 6], F32, tag=f"st6{s}")
                aggr = saves.tile([128, GBLK, 2], F32, tag=f"ag{i}{s}")
                for b in range(2):
                    nc.vector.bn_stats(out=st6[:, b, :], in_=A16[:, b, :])
                yield
                for b in range(2, GBLK):
                    nc.vector.bn_stats(out=st6[:, b, :], in_=A16[:, b, :])
                for b in range(2):
                    nc.vector.bn_aggr(out=aggr[:, b, :], in_=st6[:, b, :])
                yield
                for b in range(2, GBLK):
                    nc.vector.bn_aggr(out=aggr[:, b, :], in_=st6[:, b, :])
                lnv = stats.tile([128, GBLK, 1], F32, tag=f"lnv{s}")
                nc.scalar.activation(
                    lnv, aggr[:, :, 1:2], AF.Ln, bias=epsb[:, :]
                )
                rstd = saves.tile([128, GBLK, 1], F32, tag=f"rs{i}{s}")
                nc.scalar.activation(rstd, lnv, AF.Exp, scale=-0.5)
                yield

                xh16 = saves.tile([128, GBLK, 128], FP16, tag=f"xh{i}{s}")
                for b in range(2):
                    nc.vector.tensor_scalar(
                        out=xh16[:, b, :], in0=A16[:, b, :],
                        scalar1=aggr[:, b, 0:1], scalar2=rstd[:, b, :],
                        op0=OP.subtract, op1=OP.mult,
                    )
                yield
                tp = zpf[s].tile([128, GBLK, 128], FP16, tag=f"zpf{s}")
                for b in range(2, GBLK):
                    nc.vector.tensor_scalar(
                        out=xh16[:, b, :], in0=A16[:, b, :],
                        scalar1=aggr[:, b, 0:1], scalar2=rstd[:, b, :],
                        op0=OP.subtract, op1=OP.mult,
                    )
                for b in range(2):
                    nc.tensor.transpose(tp[:, b, :], xh16[:, b, :], ident[:, :])
                yield
                for b in range(2, GBLK):
                    nc.tensor.transpose(tp[:, b, :], xh16[:, b, :], ident[:, :])
                Xst16 = work.tile([128, GBLK, 128], FP16, tag=f"xst{s}")
                nc.vector.tensor_scalar(
                    out=Xst16, in0=tp, scalar1=1.0, scalar2=None, op0=OP.mult
                )
                out.append((A16, aggr, rstd, xh16))
                yield

            zp = zpf[s].tile([128, GBLK, 128], F32, tag=f"zpf{s}")
            nc.tensor.matmul(
                zp[:, :, :], ones1[:, :], cbs[:, 4, :],
                start=True, stop=False, skip_group_check=True,
            )
            for b in range(GBLK):
                nc.tensor.matmul(
                    zp[:, b, :], Xst16[:, b, :], wfs[:, 4, :],
                    start=False, stop=True, skip_group_check=True,
                )
            yo = io.tile([128, GBLK, 128], F32, tag=f"yout{s}", bufs=1)
            nc.scalar.copy(yo, zp)
            # issue the store from ACT (the producer) so the DMA wait is
            # satisfied by construction and never parks the SP sequencer
            nc.scalar.dma_start(out=dpv[g], in_=yo)
            yield

        def emit_bwd(g, s, sv):
            wf = io.tile([128, GBLK, 128], F32, tag=f"win{s}")
            nc.sync.dma_start(out=wf, in_=wv[g])
            tpe = zpb[s].tile([128, GBLK, 128], F32, tag=f"zpb{s}")
            for b in range(GBLK):
                nc.tensor.transpose(tpe[:, b, :], wf[:, b, :], ident32[:, :])
            Gst32 = work.tile([128, GBLK, 128], F32, tag=f"gst32{s}")
            nc.scalar.copy(Gst32, tpe)
            yield

            Gst16 = None
            for i in range(L - 1, -1, -1):
                A16, aggr, rstd, xh16 = sv[i]
                # u = exp(-A) = 1 - sigmoid of the pre-softplus input;
                # independent of the matmul chain, issue early.
                u16 = work.tile([128, GBLK, 128], FP16, tag=f"u{s}", bufs=2)
                nc.scalar.activation(u16, A16, AF.Exp, scale=-1.0)

                gp = zpb[s].tile([128, GBLK, 128], F32, tag=f"zpb{s}")
                if i == L - 1:
                    for b in range(GBLK):
                        nc.tensor.matmul(
                            gp[:, b, :], Gst32[:, b, :], wb0[:, :],
                            start=True, stop=True,
                        )
                else:
                    for b in range(GBLK):
                        nc.tensor.matmul(
                            gp[:, b, :], Gst16[:, b, :], wbs[:, 3 - i, :],
                            start=True, stop=True,
                        )
                gd16 = work.tile([128, GBLK, 128], FP16, tag=f"gd{s}", bufs=2)
                nc.scalar.copy(gd16, gp)
                yield

                # srn = rstd*u - rstd (indep of gd) interleaved with the
                # m2 = sum(xhat*gd) reduction (indep of u) so the DVE wait
                # queue never fills with one not-ready dependency group.
                srn = work.tile([128, GBLK, 128], FP16, tag=f"srn{s}", bufs=2)
                pr = work.tile([128, GBLK, 128], FP16, tag=f"pr{s}")
                m2 = stats.tile([128, GBLK, 1], F32, tag=f"m2{s}")
                for b in range(2):
                    nc.gpsimd.tensor_scalar(
                        out=srn[:, b, :], in0=u16[:, b, :],
                        scalar1=rstd[:, b, :], scalar2=rstd[:, b, :],
                        op0=OP.mult, op1=OP.subtract,
                    )
                    nc.vector.scalar_tensor_tensor(
                        out=pr[:, b, :], in0=xh16[:, b, :], scalar=1.0 / H,
                        in1=gd16[:, b, :], op0=OP.mult, op1=OP.mult,
                        accum_out=m2[:, b, :],
                    )
                yield
                for b in range(2, GBLK):
                    nc.gpsimd.tensor_scalar(
                        out=srn[:, b, :], in0=u16[:, b, :],
                        scalar1=rstd[:, b, :], scalar2=rstd[:, b, :],
                        op0=OP.mult, op1=OP.subtract,
                    )
                    nc.vector.scalar_tensor_tensor(
                        out=pr[:, b, :], in0=xh16[:, b, :], scalar=1.0 / H,
                        in1=gd16[:, b, :], op0=OP.mult, op1=OP.mult,
                        accum_out=m2[:, b, :],
                    )
                yield

                # xm = xhat*q (4x tensor_scalar), dxn = xm - gd
                xm = work.tile([128, GBLK, 128], FP16, tag=f"xm{s}")
                for b in range(2):
                    nc.vector.tensor_scalar(
                        out=xm[:, b, :], in0=xh16[:, b, :],
                        scalar1=m2[:, b, :], scalar2=None, op0=OP.mult,
                    )
                yield
                for b in range(2, GBLK):
                    nc.vector.tensor_scalar(
                        out=xm[:, b, :], in0=xh16[:, b, :],
                        scalar1=m2[:, b, :], scalar2=None, op0=OP.mult,
                    )
                dxn = work.tile([128, GBLK, 128], FP16, tag=f"dxn{s}", bufs=2)
                nc.vector.tensor_tensor(
                    out=dxn, in0=xm, in1=gd16, op=OP.subtract
                )
                yield
                # dz = dxn * srn = (gd - xhat*m) * rstd * sigmoid
                dz = work.tile([128, GBLK, 128], FP16, tag=f"dz{s}", bufs=2)
                nc.vector.tensor_tensor(out=dz, in0=dxn, in1=srn, op=OP.mult)
                yield
                tp = zpb[s].tile([128, GBLK, 128], FP16, tag=f"zpb{s}")
                for b in range(2):
                    nc.tensor.transpose(tp[:, b, :], dz[:, b, :], ident[:, :])
                yield
                for b in range(2, GBLK):
                    nc.tensor.transpose(tp[:, b, :], dz[:, b, :], ident[:, :])
                Gst16 = work.tile([128, GBLK, 128], FP16, tag=f"gst{s}")
                nc.vector.tensor_scalar(
                    out=Gst16, in0=tp, scalar1=1.0, scalar2=None, op0=OP.mult
                )
                yield

            gp = zpb[s].tile([128, GBLK, 128], F32, tag=f"zpb{s}")
            for b in range(GBLK):
                nc.tensor.matmul(
                    gp[:, b, :], Gst16[:, b, :], wbs[:, 4, :],
                    start=True, stop=True,
                )
            dwo = io.tile([128, GBLK, 128], F32, tag=f"dwout{s}", bufs=1)
            nc.scalar.copy(dwo, gp)
            nc.scalar.dma_start(out=dwv[g], in_=dwo)
            yield

        # Free-running phase-staggered pipeline: each stream s processes
        # groups s, s+NS, ... as one continuous fwd->bwd chain; streams are
        # primed with an emission-offset so their phases stay staggered and
        # every engine always sees ready work from some stream.
        assert NG % NS == 0

        def stream_gen(s):
            for g in range(s, NG, NS):
                sv = []
                yield from emit_fwd(g, s, sv)
                yield from emit_bwd(g, s, sv)

        gens = [iter(stream_gen(s)) for s in range(NS)]
        live = []
        PRIME = 6  # chunks of head-start between adjacent streams
        for s in range(NS):
            live.append(gens[s])
            for it in list(live):
                for _ in range(PRIME if it is gens[s] else 1):
                    try:
                        next(it)
                    except StopIteration:
                        if it in live:
                            live.remove(it)
                        break
        while live:
            for it in list(live):
                try:
                    next(it)
                except StopIteration:
                    live.remove(it)


def _host_precompute(t, W_in, b_in, fw, fb, gamma, beta, Wl, bl, W_out, b_out):
    t = np.asarray(t, dtype=np.float32).reshape(-1)[0]
    s = np.sin(t * np.asarray(fw, np.float32) + np.asarray(fb, np.float32))  # [L, H]
    Wl = np.asarray(Wl, np.float32)
    gamma = np.asarray(gamma, np.float32)
    beta = np.asarray(beta, np.float32)
    bl = np.asarray(bl, np.float32)
    W_in = np.asarray(W_in, np.float32)
    W_out = np.asarray(W_out, np.float32)
    b_in = np.asarray(b_in, np.float32)
    b_out = np.asarray(b_out, np.float32)

    Wg = [Wl[i] * gamma[i][None, :] for i in range(L)]          # [H, H]
    bg = [bl[i] + Wl[i] @ beta[i] for i in range(L)]            # [H]

    # fuse h4->y: y = xhat3 @ (W_out@Wg3).T + (b_out + W_out@bg3)
    M2 = (W_out.astype(np.float64) @ Wg[L - 1].astype(np.float64)).astype(np.float32)
    c = np.zeros((5, 128), np.float32)
    c[0] = b_in + s[0]
    for i in range(1, L):
        c[i] = bg[i - 1] + s[i]
    c[4] = b_out + W_out @ bg[L - 1]
    WF = np.stack(
        [W_in.T] + [Wg[i].T for i in range(L - 1)] + [M2.T], axis=0
    )  # [5, K, N]
    Wc = [Wg[i] - Wg[i].mean(axis=1, keepdims=True) for i in range(L - 1)]
    M2n = -M2
    M2c = M2n - M2n.mean(axis=1, keepdims=True)
    WB = np.stack([M2c, Wc[2], Wc[1], Wc[0], W_in], axis=0)

    WF16 = np.ascontiguousarray(np.transpose(WF, (1, 0, 2))).astype(np.float16)
    WF0 = np.ascontiguousarray(W_in.T)  # [K, N] f32
    WB16 = np.ascontiguousarray(np.transpose(WB, (1, 0, 2))).astype(np.float16)
    WB0 = np.ascontiguousarray(M2c)  # f32
    GBLK = 4
    CB = np.tile(c, (1, GBLK)).astype(np.float16)[None, :, :]  # [1, 5, GBLK*128]
    CB0 = np.tile(c[0:1, :], (1, GBLK)).astype(np.float32)     # [1, GBLK*128]
    ONES = np.ones((1, 128), np.float16)
    ONES32 = np.ones((1, 128), np.float32)
    EYE = np.eye(128, dtype=np.float16)
    EYE32 = np.eye(128, dtype=np.float32)
    return WF16, WF0, WB16, WB0, CB, CB0, ONES, ONES32, EYE, EYE32


_NC_CACHE = {}


def _get_nc(R, GBLK):
    key = (R, GBLK)
    if key not in _NC_CACHE:
        nc = bacc.Bacc("TRN2")
        _emit(nc, R, GBLK)
        nc.finalize()
        _NC_CACHE[key] = nc
    return _NC_CACHE[key]


def _run(p, w, consts, R, GBLK, n_cores):
    WF16, WF0, WB16, WB0, CB, CB0, ONES, ONES32, EYE, EYE32 = consts
    nc = _get_nc(R, GBLK)
    in_maps = []
    for k in range(n_cores):
        in_maps.append(
            {
                "p": np.ascontiguousarray(p[k * R : (k + 1) * R]),
                "w": np.ascontiguousarray(w[k * R : (k + 1) * R]),
                "wf": WF16,
                "wf0": WF0,
                "wb": WB16,
                "wb0": WB0,
                "cb": CB,
                "cb0": CB0,
                "ones1": ONES,
                "ones1_32": ONES32,
                "ident": EYE,
                "ident32": EYE32,
            }
        )
    res = run_bass_kernel_spmd(nc, in_maps, core_ids=list(range(n_cores)))
    dp = np.concatenate([r["dp"] for r in res.results], axis=0)
    dw = np.concatenate([r["dw"] for r in res.results], axis=0)
    return dp, dw


def kernel(t, p, w, W_in, b_in, fw, fb, gamma, beta, Wl, bl, W_out, b_out):
    consts = _host_precompute(
        t, W_in, b_in, fw, fb, gamma, beta, Wl, bl, W_out, b_out
    )
    p = np.asarray(p, np.float32)
    w = np.asarray(w, np.float32)
    R = p.shape[0] // NCORES
    dp, dw = _run(p, w, consts, R, GBLK=4, n_cores=NCORES)
    return dp, dw
